# revision 2
# baseline (speedup 1.0000x reference)
"""Sliding-window causal GQA self-attention (B=2, T=2048, 16 q-heads, 4 kv-heads,
head_dim=128, window=1024) on 8 trn2 NeuronCores.

Sharding: core = (batch b, kv-group g) -> 4 query heads + 1 kv head, full T.
Wo is row-parallel; each core emits a [T, 2048] partial that the host sums per
batch (the unshard step for the row-parallel layout).

Device dataflow (all matmuls float32r, free-dim >= 256 for full PE rate):
  phase 1: qT/kT/vT projections (transposed via lhsT=W chunks, rhs=x^T chunks),
           RoPE (half-swap DMA + [c;c], [s;-s] tables), RMS-norm via ACT-square +
           all-ones-matmul replicated sum, sigmoid-gated ve add into v,
           PE-transpose of v^T into natural V for the PV matmul.
  phase 2: S^T = K^T.T @ Q^T per 128-key block x 256-query super; ACT exp
           (scale fused); 0/1 triangle masks for window edges; PV and
           all-ones rowsum accumulated in PSUM; normalize on evacuation
           (y^T overwrites the dead q^T slice).
  phase 3: out[t, o] = sum_h yT_h^T @ Wo_h, Wo streamed per 512-col slice.
"""

import numpy as np

B, T, E = 2, 2048, 2048
NH, NKV, HD = 16, 4, 128
GATE_C = 32
WIN = 1024
EPS = 1e-6
NE = E // 128          # 16 contraction chunks
TC = 256               # phase-1 token chunk (= q-super width)
NTC = T // TC          # 8
NKB = T // 128         # 16 key blocks
SCALE = 1.0 / np.sqrt(HD)

_CACHE = {}


def _build_program():
    import concourse.bacc as bacc
    import concourse.mybir as mybir
    import concourse.tile as tile

    F32, F32R = mybir.dt.float32, mybir.dt.float32r
    AF = mybir.ActivationFunctionType
    OP = mybir.AluOpType

    nc = bacc.Bacc("TRN2", target_bir_lowering=False, debug=False, num_devices=8)

    xT = nc.dram_tensor("xT", [E, T], F32, kind="ExternalInput")
    veT = nc.dram_tensor("veT", [HD, T], F32, kind="ExternalInput")
    crep = nc.dram_tensor("crep", [128, T], F32, kind="ExternalInput")
    ssgn = nc.dram_tensor("ssgn", [128, T], F32, kind="ExternalInput")
    wq = nc.dram_tensor("wq", [E, 512], F32, kind="ExternalInput")
    wk = nc.dram_tensor("wk", [E, HD], F32, kind="ExternalInput")
    wv = nc.dram_tensor("wv", [E, HD], F32, kind="ExternalInput")
    wg = nc.dram_tensor("wg", [GATE_C, 128], F32, kind="ExternalInput")
    wo = nc.dram_tensor("wo", [512, E], F32, kind="ExternalInput")
    m_in = nc.dram_tensor("m_in", [4, 128, 256], F32, kind="ExternalInput")
    ones_in = nc.dram_tensor("ones_in", [128, 128], F32, kind="ExternalInput")
    eye_in = nc.dram_tensor("eye_in", [128, 128], F32, kind="ExternalInput")
    out = nc.dram_tensor("out", [T, E], F32, kind="ExternalOutput")

    with tile.TileContext(nc) as tc:
        from contextlib import ExitStack
        with ExitStack() as ctx:
            cst = ctx.enter_context(tc.tile_pool(name="cst", bufs=1))
            wts = ctx.enter_context(tc.tile_pool(name="wts", bufs=1))
            xtp = ctx.enter_context(tc.tile_pool(name="xtp", bufs=2))
            csl = ctx.enter_context(tc.tile_pool(name="csl", bufs=2))
            res = ctx.enter_context(tc.tile_pool(name="res", bufs=1))
            wk1 = ctx.enter_context(tc.tile_pool(name="wk1", bufs=2))
            wk2 = ctx.enter_context(tc.tile_pool(name="wk2", bufs=2))
            ptp = ctx.enter_context(tc.tile_pool(name="ptp", bufs=3))
            wop = ctx.enter_context(tc.tile_pool(name="wop", bufs=2))
            stg = ctx.enter_context(tc.tile_pool(name="stg", bufs=2))
            p_q = ctx.enter_context(tc.tile_pool(name="p_q", bufs=2, space="PSUM"))
            p_kv = ctx.enter_context(tc.tile_pool(name="p_kv", bufs=1, space="PSUM"))
            p_sm = ctx.enter_context(tc.tile_pool(name="p_sm", bufs=1, space="PSUM"))
            p_s = ctx.enter_context(tc.tile_pool(name="p_s", bufs=2, space="PSUM"))
            p_or = ctx.enter_context(tc.tile_pool(name="p_or", bufs=2, space="PSUM"))

            # ---- constants ----
            masks_sb = cst.tile([128, 4, 256], F32, tag="masks")
            ones_sb = cst.tile([128, 128], F32R, tag="ones")
            eye_sb = cst.tile([128, 128], F32R, tag="eye")
            eps_sb = cst.tile([128, 1], F32, tag="eps")
            nc.gpsimd.dma_start(out=masks_sb, in_=m_in.rearrange("m p f -> p m f"))
            nc.gpsimd.dma_start(out=ones_sb, in_=ones_in[:].bitcast(F32R))
            nc.gpsimd.dma_start(out=eye_sb, in_=eye_in[:].bitcast(F32R))
            nc.vector.memset(eps_sb, EPS)

            # ---- weights (resident) ----
            wq_sb = wts.tile([128, NE, 512], F32R, tag="wq")
            wk_sb = wts.tile([128, NE, HD], F32R, tag="wk")
            wv_sb = wts.tile([128, NE, HD], F32R, tag="wv")
            wg_sb = wts.tile([GATE_C, 128], F32R, tag="wg")
            nc.gpsimd.dma_start(out=wq_sb, in_=wq.rearrange("(e k) d -> k e d", k=128).bitcast(F32R))
            nc.gpsimd.dma_start(out=wk_sb, in_=wk.rearrange("(e k) d -> k e d", k=128).bitcast(F32R))
            nc.gpsimd.dma_start(out=wv_sb, in_=wv.rearrange("(e k) d -> k e d", k=128).bitcast(F32R))
            nc.gpsimd.dma_start(out=wg_sb, in_=wg[:].bitcast(F32R))

            # ---- persistent results (yT overwrites qT slices in phase 2) ----
            qyT_sb = res.tile([128, 4, T], F32R, tag="qyT")
            kT_sb = res.tile([128, T], F32R, tag="kT")
            vn_sb = res.tile([128, NKB, HD], F32R, tag="vn")

            # ================= phase 1: projections + rope + rms + gate =========
            for tcix in range(NTC):
                ts = tcix * TC
                xt = xtp.tile([128, NE, TC], F32R, tag="xt")
                nc.gpsimd.dma_start(
                    out=xt,
                    in_=xT.rearrange("(e k) t -> k e t", k=128)[:, :, ts:ts + TC].bitcast(F32R),
                )
                c_sl = csl.tile([128, TC], F32, tag="c_sl")
                s_sl = csl.tile([128, TC], F32, tag="s_sl")
                ve_sl = csl.tile([HD, TC], F32, tag="ve_sl")
                nc.gpsimd.dma_start(out=c_sl, in_=crep[:, ts:ts + TC])
                nc.gpsimd.dma_start(out=s_sl, in_=ssgn[:, ts:ts + TC])
                nc.gpsimd.dma_start(out=ve_sl, in_=veT[:, ts:ts + TC])

                # gate: replicated [128, TC] = Wg_rep.T @ x[:, :32].T ; sigmoid
                g_ps = p_sm.tile([128, TC], F32, tag="small")
                nc.tensor.matmul(g_ps, wg_sb, xt[0:GATE_C, 0, :], start=True, stop=True)
                g_rep = wk2.tile([128, TC], F32, tag="grep")
                nc.scalar.activation(g_rep, g_ps, AF.Sigmoid)

                srcs = [("q", h) for h in range(4)] + [("k", 0)]
                for kind, h in srcs:
                    if kind == "q":
                        ps = p_q.tile([128, TC], F32, tag="q")
                        for e in range(NE):
                            nc.tensor.matmul(ps, wq_sb[:, e, h * 128:(h + 1) * 128],
                                             xt[:, e, :], start=(e == 0), stop=(e == NE - 1))
                    else:
                        ps = p_kv.tile([128, TC], F32, tag="kv")
                        for e in range(NE):
                            nc.tensor.matmul(ps, wk_sb[:, e, :], xt[:, e, :],
                                             start=(e == 0), stop=(e == NE - 1))

                    # evacuate + square (ACT), swap halves (DMA), rope + rms (DVE)
                    qraw = wk1.tile([128, TC], F32, tag="qraw")
                    nc.scalar.copy(qraw, ps)
                    sq = wk1.tile([128, TC], F32R, tag="sq")
                    nc.scalar.square(sq, ps)
                    ss_ps = p_sm.tile([128, TC], F32, tag="small")
                    nc.tensor.matmul(ss_ps, ones_sb, sq, start=True, stop=True)
                    rrms = wk2.tile([128, TC], F32, tag="rrms")
                    nc.scalar.activation(rrms, ss_ps, AF.Sqrt, bias=eps_sb, scale=1.0 / HD)
                    nc.vector.reciprocal(rrms, rrms)
                    qsw = wk1.tile([128, TC], F32, tag="qsw")
                    nc.gpsimd.dma_start(out=qsw[0:64, :], in_=qraw[64:128, :])
                    nc.gpsimd.dma_start(out=qsw[64:128, :], in_=qraw[0:64, :])
                    tA = wk1.tile([128, TC], F32, tag="tA")
                    tB = wk1.tile([128, TC], F32, tag="tB")
                    nc.vector.tensor_mul(tA, qraw, c_sl)
                    nc.vector.tensor_mul(tB, qsw, s_sl)
                    nc.vector.tensor_add(tA, tA, tB)
                    dest = qyT_sb[:, h, ts:ts + TC] if kind == "q" else kT_sb[:, ts:ts + TC]
                    nc.vector.tensor_mul(dest, tA, rrms)

                # v: projection + gated ve + transpose to natural layout
                ps_v = p_kv.tile([128, TC], F32, tag="kv")
                for e in range(NE):
                    nc.tensor.matmul(ps_v, wv_sb[:, e, :], xt[:, e, :],
                                     start=(e == 0), stop=(e == NE - 1))
                tv = wk1.tile([128, TC], F32, tag="tA")
                nc.gpsimd.tensor_tensor(tv, ve_sl, g_rep, OP.mult)
                vt = wk1.tile([128, TC], F32R, tag="tB")
                nc.vector.scalar_tensor_tensor(vt, tv, 2.0, ps_v, OP.mult, OP.add)
                for tb in range(TC // 128):
                    tp_ps = p_sm.tile([128, 128], F32R, tag="small")
                    nc.tensor.transpose(tp_ps, vt[:, tb * 128:(tb + 1) * 128], eye_sb)
                    nc.vector.tensor_copy(vn_sb[:, tcix * 2 + tb, :], tp_ps)

            # ================= phase 2: windowed attention =====================
            for h in range(4):
                for qs in range(NTC):
                    q0 = qs * TC
                    kb0 = max(0, 2 * qs - 8)
                    kb1 = 2 * qs + 2
                    o_ps = p_or.tile([128, TC], F32, tag="or")
                    r_ps = p_or.tile([128, TC], F32, tag="or")
                    for j in range(kb0, kb1, 2):
                        s_ps = p_s.tile([128, 512], F32, tag="s")
                        pt = ptp.tile([128, 512], F32R, tag="pt")
                        for u in range(2):
                            kb = j + u
                            nc.tensor.matmul(s_ps[:, u * 256:(u + 1) * 256],
                                             kT_sb[:, kb * 128:(kb + 1) * 128],
                                             qyT_sb[:, h, q0:q0 + TC],
                                             start=True, stop=True)
                        nc.scalar.activation(pt, s_ps, AF.Exp, scale=float(SCALE))
                        for u in range(2):
                            kb = j + u
                            mi = None
                            if kb == 2 * qs:
                                mi = 0
                            elif kb == 2 * qs + 1:
                                mi = 1
                            elif qs >= 4 and kb == kb0:
                                mi = 2
                            elif qs >= 4 and kb == kb0 + 1:
                                mi = 3
                            ptu = pt[:, u * 256:(u + 1) * 256]
                            if mi is not None:
                                nc.gpsimd.tensor_tensor(ptu, ptu, masks_sb[:, mi, :], OP.mult)
                            nc.tensor.matmul(o_ps, vn_sb[:, kb, :], ptu,
                                             start=(kb == kb0), stop=(kb == kb1 - 1))
                            nc.tensor.matmul(r_ps, ones_sb, ptu,
                                             start=(kb == kb0), stop=(kb == kb1 - 1))
                    rr = wk2.tile([128, TC], F32, tag="rr")
                    nc.vector.reciprocal(rr, r_ps)
                    # y^T overwrites the (now dead) q^T slice
                    nc.vector.tensor_mul(qyT_sb[:, h, q0:q0 + TC], o_ps, rr)

            # ================= phase 3: out = y @ Wo (row-parallel partial) ====
            for os_ in range(4):
                wo_sl = wop.tile([128, 4, 512], F32R, tag="wo")
                nc.gpsimd.dma_start(
                    out=wo_sl,
                    in_=wo.rearrange("(h d) o -> d h o", d=128)[:, :, os_ * 512:(os_ + 1) * 512].bitcast(F32R),
                )
                for tt in range(T // 128):
                    po = p_or.tile([128, 512], F32, tag="or")
                    for h in range(4):
                        nc.tensor.matmul(po, qyT_sb[:, h, tt * 128:(tt + 1) * 128],
                                         wo_sl[:, h, :], start=(h == 0), stop=(h == 3))
                    stage = stg.tile([128, 512], F32, tag="stage")
                    nc.vector.tensor_copy(stage, po)
                    nc.gpsimd.dma_start(
                        out=out[tt * 128:(tt + 1) * 128, os_ * 512:(os_ + 1) * 512],
                        in_=stage)

    nc.compile()
    return nc


def _masks():
    jj = np.arange(128)[:, None]
    ii = np.arange(128)[None, :]
    tri_d = (jj <= ii).astype(np.float32)   # diag block: keep j <= i
    tri_f = (jj >= ii).astype(np.float32)   # far block: keep j >= i - WIN
    one = np.ones((128, 128), np.float32)
    zero = np.zeros((128, 128), np.float32)
    m0 = np.concatenate([tri_d, one], 1)
    m1 = np.concatenate([zero, tri_d], 1)
    m2 = np.concatenate([tri_f, zero], 1)
    m3 = np.concatenate([one, tri_f], 1)
    return np.stack([m0, m1, m2, m3])


def kernel(**inputs):
    from concourse.bass_utils import run_bass_kernel_spmd

    if "nc" not in _CACHE:
        _CACHE["nc"] = _build_program()
    nc = _CACHE["nc"]

    x = np.asarray(inputs["x"], np.float32)
    ve = np.asarray(inputs["ve"], np.float32)
    cos = np.asarray(inputs["cos"], np.float32)
    sin = np.asarray(inputs["sin"], np.float32)
    Wq = np.asarray(inputs["Wq"], np.float32)
    Wk = np.asarray(inputs["Wk"], np.float32)
    Wv = np.asarray(inputs["Wv"], np.float32)
    Wo = np.asarray(inputs["Wo"], np.float32)
    Wg = np.asarray(inputs["Wg"], np.float32)

    crep = np.ascontiguousarray(np.concatenate([cos.T, cos.T], 0))
    ssgn = np.ascontiguousarray(np.concatenate([sin.T, -sin.T], 0))
    masks = _masks()
    ones128 = np.ones((128, 128), np.float32)
    eye128 = np.eye(128, dtype=np.float32)

    in_maps = []
    for c in range(8):
        b, g = divmod(c, 4)
        in_maps.append({
            "xT": np.ascontiguousarray(x[b].T),
            "veT": np.ascontiguousarray(ve[b, :, g * HD:(g + 1) * HD].T),
            "crep": crep,
            "ssgn": ssgn,
            "wq": np.ascontiguousarray(Wq[:, g * 512:(g + 1) * 512]),
            "wk": np.ascontiguousarray(Wk[:, g * HD:(g + 1) * HD]),
            "wv": np.ascontiguousarray(Wv[:, g * HD:(g + 1) * HD]),
            "wg": np.ascontiguousarray(np.repeat(Wg[:, g:g + 1], 128, 1)),
            "wo": np.ascontiguousarray(Wo[g * 512:(g + 1) * 512, :]),
            "m_in": masks,
            "ones_in": ones128,
            "eye_in": eye128,
        })

    res = run_bass_kernel_spmd(nc, in_maps, core_ids=list(range(8)))
    parts = [res.results[c]["out"] for c in range(8)]
    out = np.stack([parts[0] + parts[1] + parts[2] + parts[3],
                    parts[4] + parts[5] + parts[6] + parts[7]])
    return out.astype(np.float32)


# revision 10
# speedup vs baseline: 1.3707x; 1.3707x over previous
"""Sliding-window causal GQA self-attention (B=2, T=2048, 16 q-heads, 4 kv-heads,
head_dim=128, window=1024) on 8 trn2 NeuronCores.

Sharding: core = (batch b, kv-group g) -> 4 query heads + 1 kv head, full T.
Wo is row-parallel; each core emits a [T, 2048] partial that the host sums per
batch (the unshard step for the row-parallel layout).

Device dataflow (all matmuls float32r, free-dim >= 256 for full PE rate):
  phase 1: qT/kT/vT projections (transposed via lhsT=W chunks, rhs=x^T chunks),
           RoPE (half-swap DMA + [c;c], [s;-s] tables), RMS-norm via ACT-square +
           all-ones-matmul replicated sum (sqrt ops paired to limit act-table
           reloads), gate sigmoid computed via Exp (shares the softmax act
           table), PE-transpose of v^T into natural V for the PV matmul.
  phase 2: S^T = K^T.T @ Q^T per 128-key block x 256-query super; ACT exp
           (scale fused); 0/1 triangle masks for window edges; PV and
           all-ones rowsum accumulated in PSUM; normalize on evacuation
           (y^T overwrites the dead q^T slice).
  phase 3: out[t, o] = sum_h yT_h^T @ Wo_h, Wo streamed per 512-col slice.
"""

import numpy as np

B, T, E = 2, 2048, 2048
NH, NKV, HD = 16, 4, 128
GATE_C = 32
WIN = 1024
EPS = 1e-6
NE = E // 128          # 16 contraction chunks
TC = 256               # phase-1 token chunk (= q-super width)
NTC = T // TC          # 8
NKB = T // 128         # 16 key blocks
SCALE = 1.0 / np.sqrt(HD)

_CACHE = {}


def _build_program():
    import concourse.bacc as bacc
    import concourse.mybir as mybir
    import concourse.tile as tile

    F32, F32R = mybir.dt.float32, mybir.dt.float32r
    AF = mybir.ActivationFunctionType
    OP = mybir.AluOpType

    nc = bacc.Bacc("TRN2", target_bir_lowering=False, debug=False, num_devices=8)

    xT = nc.dram_tensor("xT", [E, T], F32, kind="ExternalInput")
    veT = nc.dram_tensor("veT", [HD, T], F32, kind="ExternalInput")
    crep = nc.dram_tensor("crep", [128, T], F32, kind="ExternalInput")
    ssgn = nc.dram_tensor("ssgn", [128, T], F32, kind="ExternalInput")
    wq = nc.dram_tensor("wq", [E, 512], F32, kind="ExternalInput")
    wk = nc.dram_tensor("wk", [E, HD], F32, kind="ExternalInput")
    wv = nc.dram_tensor("wv", [E, HD], F32, kind="ExternalInput")
    wg = nc.dram_tensor("wg", [GATE_C, 128], F32, kind="ExternalInput")
    wo = nc.dram_tensor("wo", [512, E], F32, kind="ExternalInput")
    m_in = nc.dram_tensor("m_in", [4, 128, 256], F32, kind="ExternalInput")
    ones_in = nc.dram_tensor("ones_in", [128, 128], F32, kind="ExternalInput")
    eye_in = nc.dram_tensor("eye_in", [128, 128], F32, kind="ExternalInput")
    out = nc.dram_tensor("out", [T, E], F32, kind="ExternalOutput")

    xT_r = xT.rearrange("(e k) t -> k e t", k=128)
    wq_r = wq.rearrange("(e k) d -> k e d", k=128)
    wk_r = wk.rearrange("(e k) d -> k e d", k=128)
    wv_r = wv.rearrange("(e k) d -> k e d", k=128)

    with tile.TileContext(nc) as tc:
        from contextlib import ExitStack
        with ExitStack() as ctx:
            cst = ctx.enter_context(tc.tile_pool(name="cst", bufs=1))
            wts = ctx.enter_context(tc.tile_pool(name="wts", bufs=1))
            xtp = ctx.enter_context(tc.tile_pool(name="xtp", bufs=2))
            csl = ctx.enter_context(tc.tile_pool(name="csl", bufs=2))
            res = ctx.enter_context(tc.tile_pool(name="res", bufs=1))
            qrp = ctx.enter_context(tc.tile_pool(name="qrp", bufs=3))
            wk1 = ctx.enter_context(tc.tile_pool(name="wk1", bufs=2))
            wk2 = ctx.enter_context(tc.tile_pool(name="wk2", bufs=2))
            ptp = ctx.enter_context(tc.tile_pool(name="ptp", bufs=3))
            wop = ctx.enter_context(tc.tile_pool(name="wop", bufs=2))
            stg = ctx.enter_context(tc.tile_pool(name="stg", bufs=4))
            p_q = ctx.enter_context(tc.tile_pool(name="p_q", bufs=2, space="PSUM"))
            p_sm = ctx.enter_context(tc.tile_pool(name="p_sm", bufs=1, space="PSUM"))
            p_s = ctx.enter_context(tc.tile_pool(name="p_s", bufs=2, space="PSUM"))
            p_or = ctx.enter_context(tc.tile_pool(name="p_or", bufs=3, space="PSUM"))

            # ---- small constants ----
            masks_sb = cst.tile([128, 4, 256], F32, tag="masks")
            ones_sb = cst.tile([128, 128], F32R, tag="ones")
            eye_sb = cst.tile([128, 128], F32R, tag="eye")
            eps_sb = cst.tile([128, 1], F32, tag="eps")
            nc.sync.dma_start(out=masks_sb, in_=m_in.rearrange("m p f -> p m f"))
            nc.sync.dma_start(out=ones_sb, in_=ones_in[:].bitcast(F32R))
            nc.sync.dma_start(out=eye_sb, in_=eye_in[:].bitcast(F32R))
            nc.vector.memset(eps_sb, EPS)

            wg_sb = wts.tile([GATE_C, 128], F32R, tag="wg")
            nc.sync.dma_start(out=wg_sb, in_=wg[:].bitcast(F32R))

            # ---- chunk-0 stream DMAs FIRST so compute starts early ----
            xt0 = xtp.tile([128, NE, TC], F32R, tag="xt")
            for e4 in range(4):
                sl = slice(e4 * 4, (e4 + 1) * 4)
                nc.sync.dma_start(out=xt0[:, sl, :], in_=xT_r[:, sl, 0:TC].bitcast(F32R))
            c0 = csl.tile([128, TC], F32, tag="c_sl")
            s0 = csl.tile([128, TC], F32, tag="s_sl")
            v0 = csl.tile([HD, TC], F32, tag="ve_sl")
            nc.sync.dma_start(out=c0, in_=crep[:, 0:TC])
            nc.sync.dma_start(out=s0, in_=ssgn[:, 0:TC])
            nc.sync.dma_start(out=v0, in_=veT[:, 0:TC])

            # ---- weights, split by e-chunk groups (interleaved queues) ----
            wq_sb = wts.tile([128, NE, 512], F32R, tag="wq")
            wk_sb = wts.tile([128, NE, HD], F32R, tag="wk")
            wv_sb = wts.tile([128, NE, HD], F32R, tag="wv")
            for e4 in range(4):
                sl = slice(e4 * 4, (e4 + 1) * 4)
                nc.sync.dma_start(out=wq_sb[:, sl, :], in_=wq_r[:, sl, :].bitcast(F32R))
                nc.sync.dma_start(out=wk_sb[:, sl, :], in_=wk_r[:, sl, :].bitcast(F32R))
                nc.sync.dma_start(out=wv_sb[:, sl, :], in_=wv_r[:, sl, :].bitcast(F32R))

            # ---- persistent results (yT overwrites qT slices in phase 2) ----
            qyT_sb = res.tile([128, 4, T], F32R, tag="qyT")
            kT_sb = res.tile([128, T], F32R, tag="kT")
            vn_sb = res.tile([128, NKB, HD], F32R, tag="vn")

            # ================= phase 1 ==========================================
            for tcix in range(NTC):
                ts = tcix * TC
                if tcix == 0:
                    xt, c_sl, s_sl, ve_sl = xt0, c0, s0, v0
                else:
                    xt = xtp.tile([128, NE, TC], F32R, tag="xt")
                    nc.sync.dma_start(out=xt, in_=xT_r[:, :, ts:ts + TC].bitcast(F32R))
                    c_sl = csl.tile([128, TC], F32, tag="c_sl")
                    s_sl = csl.tile([128, TC], F32, tag="s_sl")
                    ve_sl = csl.tile([HD, TC], F32, tag="ve_sl")
                    nc.sync.dma_start(out=c_sl, in_=crep[:, ts:ts + TC])
                    nc.sync.dma_start(out=s_sl, in_=ssgn[:, ts:ts + TC])
                    nc.sync.dma_start(out=ve_sl, in_=veT[:, ts:ts + TC])

                # gate via exp: g = 1/(1+exp(-u)); the 2x is folded in the STT
                g_ps = p_sm.tile([128, TC], F32, tag="small")
                nc.tensor.matmul(g_ps, wg_sb, xt[0:GATE_C, 0, :], start=True, stop=True)
                g_rep = wk2.tile([128, TC], F32, tag="grep")
                nc.scalar.activation(g_rep, g_ps, AF.Exp, scale=-1.0)
                nc.vector.tensor_scalar_add(g_rep, g_rep, 1.0)
                nc.vector.reciprocal(g_rep, g_rep)

                # projections + rms + rope, per source
                srcs = [("q", 0), ("q", 1), ("q", 2), ("q", 3), ("k", 0)]
                for i, (kind, h) in enumerate(srcs):
                    ps = p_q.tile([128, TC], F32, tag="q")
                    w_sb = wq_sb if kind == "q" else wk_sb
                    for e in range(NE):
                        lhs = w_sb[:, e, h * 128:(h + 1) * 128] if kind == "q" else w_sb[:, e, :]
                        nc.tensor.matmul(ps, lhs, xt[:, e, :],
                                         start=(e == 0), stop=(e == NE - 1))
                    qraw = qrp.tile([128, TC], F32, tag="qraw")
                    nc.vector.tensor_copy(qraw, ps)
                    sq = wk1.tile([128, TC], F32R, tag="sq")
                    nc.scalar.square(sq, ps)
                    ss_ps = p_sm.tile([128, TC], F32, tag="small")
                    nc.tensor.matmul(ss_ps, ones_sb, sq, start=True, stop=True)
                    rrms = wk2.tile([128, TC], F32, tag="rrms")
                    nc.scalar.activation(rrms, ss_ps, AF.Sqrt, bias=eps_sb, scale=1.0 / HD)
                    nc.vector.reciprocal(rrms, rrms)
                    qsw = wk1.tile([128, TC], F32, tag="qsw")
                    nc.sync.dma_start(out=qsw[0:64, :], in_=qraw[64:128, :])
                    nc.sync.dma_start(out=qsw[64:128, :], in_=qraw[0:64, :])
                    tA = wk1.tile([128, TC], F32, tag="tA")
                    tB = wk1.tile([128, TC], F32, tag="tB")
                    nc.vector.tensor_mul(tA, ps, c_sl)
                    nc.vector.tensor_mul(tB, qsw, s_sl)
                    nc.vector.tensor_add(tA, tA, tB)
                    dest = qyT_sb[:, h, ts:ts + TC] if kind == "q" else kT_sb[:, ts:ts + TC]
                    nc.vector.tensor_mul(dest, tA, rrms)

                # v: projection + gated ve + transpose to natural layout
                ps_v = p_q.tile([128, TC], F32, tag="q")
                for e in range(NE):
                    nc.tensor.matmul(ps_v, wv_sb[:, e, :], xt[:, e, :],
                                     start=(e == 0), stop=(e == NE - 1))
                tv = wk1.tile([128, TC], F32, tag="tA")
                nc.vector.tensor_mul(tv, ve_sl, g_rep)
                vt = wk1.tile([128, TC], F32R, tag="tB")
                nc.vector.scalar_tensor_tensor(vt, tv, 2.0, ps_v, OP.mult, OP.add)
                for tb in range(TC // 128):
                    tp_ps = p_sm.tile([128, 128], F32R, tag="small")
                    nc.tensor.transpose(tp_ps, vt[:, tb * 128:(tb + 1) * 128], eye_sb)
                    nc.vector.tensor_copy(vn_sb[:, tcix * 2 + tb, :], tp_ps)

            # ================= phase 2: windowed attention =====================
            for h in range(4):
                for qs in range(NTC):
                    q0 = qs * TC
                    kb0 = max(0, 2 * qs - 8)
                    kb1 = 2 * qs + 2
                    o_ps = p_or.tile([128, TC], F32, tag="or")
                    r_ps = p_or.tile([128, TC], F32, tag="or")
                    for j in range(kb0, kb1, 2):
                        s_ps = p_s.tile([128, 512], F32, tag="s")
                        pt = ptp.tile([128, 512], F32R, tag="pt")
                        for u in range(2):
                            kb = j + u
                            nc.tensor.matmul(s_ps[:, u * 256:(u + 1) * 256],
                                             kT_sb[:, kb * 128:(kb + 1) * 128],
                                             qyT_sb[:, h, q0:q0 + TC],
                                             start=True, stop=True)
                        nc.scalar.activation(pt, s_ps, AF.Exp, scale=float(SCALE))
                        for u in range(2):
                            kb = j + u
                            mi = None
                            if kb == 2 * qs:
                                mi = 0
                            elif kb == 2 * qs + 1:
                                mi = 1
                            elif qs >= 4 and kb == kb0:
                                mi = 2
                            elif qs >= 4 and kb == kb0 + 1:
                                mi = 3
                            ptu = pt[:, u * 256:(u + 1) * 256]
                            if mi is not None:
                                nc.vector.tensor_tensor(ptu, ptu, masks_sb[:, mi, :], OP.mult)
                            nc.tensor.matmul(o_ps, vn_sb[:, kb, :], ptu,
                                             start=(kb == kb0), stop=(kb == kb1 - 1))
                            nc.tensor.matmul(r_ps, ones_sb, ptu,
                                             start=(kb == kb0), stop=(kb == kb1 - 1))
                    rr = wk2.tile([128, TC], F32, tag="rr")
                    nc.vector.reciprocal(rr, r_ps)
                    # y^T overwrites the (now dead) q^T slice
                    nc.vector.tensor_mul(qyT_sb[:, h, q0:q0 + TC], o_ps, rr)

            # ================= phase 3: out = y @ Wo (row-parallel partial) ====
            for os_ in range(4):
                wo_sl = wop.tile([128, 4, 512], F32R, tag="wo")
                nc.sync.dma_start(
                    out=wo_sl,
                    in_=wo.rearrange("(h d) o -> d h o", d=128)[:, :, os_ * 512:(os_ + 1) * 512].bitcast(F32R),
                )
                for tt in range(T // 128):
                    pool3, tag3 = (p_s, "s") if tt % 2 == 0 else (p_or, "or")
                    po = pool3.tile([128, 512], F32, tag=tag3)
                    for h in range(4):
                        nc.tensor.matmul(po, qyT_sb[:, h, tt * 128:(tt + 1) * 128],
                                         wo_sl[:, h, :], start=(h == 0), stop=(h == 3))
                    stage = stg.tile([128, 512], F32, tag="stage")
                    if tt % 2 == 0:
                        nc.vector.tensor_copy(stage, po)
                    else:
                        nc.scalar.copy(stage, po)
                    nc.sync.dma_start(
                        out=out[tt * 128:(tt + 1) * 128, os_ * 512:(os_ + 1) * 512],
                        in_=stage)

    nc.compile()
    return nc


def _masks():
    jj = np.arange(128)[:, None]
    ii = np.arange(128)[None, :]
    tri_d = (jj <= ii).astype(np.float32)   # diag block: keep j <= i
    tri_f = (jj >= ii).astype(np.float32)   # far block: keep j >= i - WIN
    one = np.ones((128, 128), np.float32)
    zero = np.zeros((128, 128), np.float32)
    m0 = np.concatenate([tri_d, one], 1)
    m1 = np.concatenate([zero, tri_d], 1)
    m2 = np.concatenate([tri_f, zero], 1)
    m3 = np.concatenate([one, tri_f], 1)
    return np.stack([m0, m1, m2, m3])


def kernel(**inputs):
    from concourse.bass_utils import run_bass_kernel_spmd

    if "nc" not in _CACHE:
        _CACHE["nc"] = _build_program()
    nc = _CACHE["nc"]

    x = np.asarray(inputs["x"], np.float32)
    ve = np.asarray(inputs["ve"], np.float32)
    cos = np.asarray(inputs["cos"], np.float32)
    sin = np.asarray(inputs["sin"], np.float32)
    Wq = np.asarray(inputs["Wq"], np.float32)
    Wk = np.asarray(inputs["Wk"], np.float32)
    Wv = np.asarray(inputs["Wv"], np.float32)
    Wo = np.asarray(inputs["Wo"], np.float32)
    Wg = np.asarray(inputs["Wg"], np.float32)

    crep = np.ascontiguousarray(np.concatenate([cos.T, cos.T], 0))
    ssgn = np.ascontiguousarray(np.concatenate([sin.T, -sin.T], 0))
    masks = _masks()
    ones128 = np.ones((128, 128), np.float32)
    eye128 = np.eye(128, dtype=np.float32)

    in_maps = []
    for c in range(8):
        b, g = divmod(c, 4)
        in_maps.append({
            "xT": np.ascontiguousarray(x[b].T),
            "veT": np.ascontiguousarray(ve[b, :, g * HD:(g + 1) * HD].T),
            "crep": crep,
            "ssgn": ssgn,
            "wq": np.ascontiguousarray(Wq[:, g * 512:(g + 1) * 512]),
            "wk": np.ascontiguousarray(Wk[:, g * HD:(g + 1) * HD]),
            "wv": np.ascontiguousarray(Wv[:, g * HD:(g + 1) * HD]),
            "wg": np.ascontiguousarray(np.repeat(Wg[:, g:g + 1], 128, 1)),
            "wo": np.ascontiguousarray(Wo[g * 512:(g + 1) * 512, :]),
            "m_in": masks,
            "ones_in": ones128,
            "eye_in": eye128,
        })

    res = run_bass_kernel_spmd(nc, in_maps, core_ids=list(range(8)))
    parts = [res.results[c]["out"] for c in range(8)]
    out = np.stack([parts[0] + parts[1] + parts[2] + parts[3],
                    parts[4] + parts[5] + parts[6] + parts[7]])
    return out.astype(np.float32)


# revision 14
# speedup vs baseline: 1.4078x; 1.0271x over previous
"""Sliding-window causal GQA self-attention (B=2, T=2048, 16 q-heads, 4 kv-heads,
head_dim=128, window=1024) on 8 trn2 NeuronCores.

Sharding: core = (batch b, kv-group g) -> 4 query heads + 1 kv head, full T.
Wo is row-parallel; each core emits a [T, 2048] partial that the host sums per
batch (the unshard step for the row-parallel layout).

Device dataflow (all matmuls float32r, free-dim >= 256 for full PE rate):
  phase 1: qT/kT/vT projections (transposed via lhsT=W chunks, rhs=x^T chunks),
           RoPE (half-swap DMA + [c;c], [s;-s] tables), RMS-norm via ACT-square +
           all-ones-matmul replicated sum (sqrt ops paired to limit act-table
           reloads), gate sigmoid computed via Exp (shares the softmax act
           table), PE-transpose of v^T into natural V for the PV matmul.
  phase 2: S^T = K^T.T @ Q^T per 128-key block x 256-query super; ACT exp
           (scale fused); 0/1 triangle masks for window edges; PV and
           all-ones rowsum accumulated in PSUM; normalize on evacuation
           (y^T overwrites the dead q^T slice).
  phase 3: out[t, o] = sum_h yT_h^T @ Wo_h, Wo streamed per 512-col slice.
"""

import numpy as np

B, T, E = 2, 2048, 2048
NH, NKV, HD = 16, 4, 128
GATE_C = 32
WIN = 1024
EPS = 1e-6
NE = E // 128          # 16 contraction chunks
TC = 256               # phase-1 token chunk (= q-super width)
NTC = T // TC          # 8
NKB = T // 128         # 16 key blocks
SCALE = 1.0 / np.sqrt(HD)

_CACHE = {}


def _build_program():
    import concourse.bacc as bacc
    import concourse.mybir as mybir
    import concourse.tile as tile

    F32, F32R = mybir.dt.float32, mybir.dt.float32r
    AF = mybir.ActivationFunctionType
    OP = mybir.AluOpType

    nc = bacc.Bacc("TRN2", target_bir_lowering=False, debug=False, num_devices=8)

    xT = nc.dram_tensor("xT", [E, T], F32, kind="ExternalInput")
    veT = nc.dram_tensor("veT", [HD, T], F32, kind="ExternalInput")
    crep = nc.dram_tensor("crep", [128, T], F32, kind="ExternalInput")
    ssgn = nc.dram_tensor("ssgn", [128, T], F32, kind="ExternalInput")
    wq = nc.dram_tensor("wq", [E, 512], F32, kind="ExternalInput")
    wk = nc.dram_tensor("wk", [E, HD], F32, kind="ExternalInput")
    wv = nc.dram_tensor("wv", [E, HD], F32, kind="ExternalInput")
    wg = nc.dram_tensor("wg", [GATE_C, 128], F32, kind="ExternalInput")
    wo = nc.dram_tensor("wo", [512, E], F32, kind="ExternalInput")
    m_in = nc.dram_tensor("m_in", [4, 128, 256], F32, kind="ExternalInput")
    ones_in = nc.dram_tensor("ones_in", [128, 128], F32, kind="ExternalInput")
    eye_in = nc.dram_tensor("eye_in", [128, 128], F32, kind="ExternalInput")
    out = nc.dram_tensor("out", [T, E], F32, kind="ExternalOutput")

    xT_r = xT.rearrange("(e k) t -> k e t", k=128)
    wq_r = wq.rearrange("(e k) d -> k e d", k=128)
    wk_r = wk.rearrange("(e k) d -> k e d", k=128)
    wv_r = wv.rearrange("(e k) d -> k e d", k=128)

    with tile.TileContext(nc) as tc:
        from contextlib import ExitStack
        with ExitStack() as ctx:
            cst = ctx.enter_context(tc.tile_pool(name="cst", bufs=1))
            wts = ctx.enter_context(tc.tile_pool(name="wts", bufs=1))
            xtp = ctx.enter_context(tc.tile_pool(name="xtp", bufs=2))
            csl = ctx.enter_context(tc.tile_pool(name="csl", bufs=3))
            res = ctx.enter_context(tc.tile_pool(name="res", bufs=1))
            qrp = ctx.enter_context(tc.tile_pool(name="qrp", bufs=5))
            wk1 = ctx.enter_context(tc.tile_pool(name="wk1", bufs=3))
            wk2 = ctx.enter_context(tc.tile_pool(name="wk2", bufs=3))
            ptp = ctx.enter_context(tc.tile_pool(name="ptp", bufs=3))
            wop = ctx.enter_context(tc.tile_pool(name="wop", bufs=2))
            stg = ctx.enter_context(tc.tile_pool(name="stg", bufs=4))
            p_q = ctx.enter_context(tc.tile_pool(name="p_q", bufs=2, space="PSUM"))
            p_sm = ctx.enter_context(tc.tile_pool(name="p_sm", bufs=1, space="PSUM"))
            p_s = ctx.enter_context(tc.tile_pool(name="p_s", bufs=2, space="PSUM"))
            p_or = ctx.enter_context(tc.tile_pool(name="p_or", bufs=3, space="PSUM"))

            # ---- small constants ----
            masks_sb = cst.tile([128, 4, 256], F32, tag="masks")
            ones_sb = cst.tile([128, 128], F32R, tag="ones")
            eye_sb = cst.tile([128, 128], F32R, tag="eye")
            eps_sb = cst.tile([128, 1], F32, tag="eps")
            nc.sync.dma_start(out=masks_sb, in_=m_in.rearrange("m p f -> p m f"))
            nc.sync.dma_start(out=ones_sb, in_=ones_in[:].bitcast(F32R))
            nc.sync.dma_start(out=eye_sb, in_=eye_in[:].bitcast(F32R))
            nc.vector.memset(eps_sb, EPS)

            wg_sb = wts.tile([GATE_C, 128], F32R, tag="wg")
            nc.sync.dma_start(out=wg_sb, in_=wg[:].bitcast(F32R))

            # ---- chunk-0 stream DMAs FIRST so compute starts early ----
            xt0 = xtp.tile([128, NE, TC], F32R, tag="xt")
            for e4 in range(4):
                sl = slice(e4 * 4, (e4 + 1) * 4)
                nc.sync.dma_start(out=xt0[:, sl, :], in_=xT_r[:, sl, 0:TC].bitcast(F32R))
            c0 = csl.tile([128, TC], F32, tag="c_sl")
            s0 = csl.tile([128, TC], F32, tag="s_sl")
            v0 = csl.tile([HD, TC], F32, tag="ve_sl")
            nc.sync.dma_start(out=c0, in_=crep[:, 0:TC])
            nc.sync.dma_start(out=s0, in_=ssgn[:, 0:TC])
            nc.sync.dma_start(out=v0, in_=veT[:, 0:TC])

            # ---- weights, split by e-chunk groups (interleaved queues) ----
            wq_sb = wts.tile([128, NE, 512], F32R, tag="wq")
            wk_sb = wts.tile([128, NE, HD], F32R, tag="wk")
            wv_sb = wts.tile([128, NE, HD], F32R, tag="wv")
            for e4 in range(4):
                sl = slice(e4 * 4, (e4 + 1) * 4)
                nc.sync.dma_start(out=wq_sb[:, sl, :], in_=wq_r[:, sl, :].bitcast(F32R))
                nc.sync.dma_start(out=wk_sb[:, sl, :], in_=wk_r[:, sl, :].bitcast(F32R))
                nc.sync.dma_start(out=wv_sb[:, sl, :], in_=wv_r[:, sl, :].bitcast(F32R))

            # ---- persistent results (yT overwrites qT slices in phase 2) ----
            qyT_sb = res.tile([128, 4, T], F32R, tag="qyT")
            kT_sb = res.tile([128, T], F32R, tag="kT")
            vn_sb = res.tile([128, NKB, HD], F32R, tag="vn")

            # ================= phase 1 ==========================================
            for tcix in range(NTC):
                ts = tcix * TC
                if tcix == 0:
                    xt, c_sl, s_sl, ve_sl = xt0, c0, s0, v0
                else:
                    xt = xtp.tile([128, NE, TC], F32R, tag="xt")
                    nc.sync.dma_start(out=xt, in_=xT_r[:, :, ts:ts + TC].bitcast(F32R))
                    c_sl = csl.tile([128, TC], F32, tag="c_sl")
                    s_sl = csl.tile([128, TC], F32, tag="s_sl")
                    ve_sl = csl.tile([HD, TC], F32, tag="ve_sl")
                    nc.sync.dma_start(out=c_sl, in_=crep[:, ts:ts + TC])
                    nc.sync.dma_start(out=s_sl, in_=ssgn[:, ts:ts + TC])
                    nc.sync.dma_start(out=ve_sl, in_=veT[:, ts:ts + TC])

                # gate via exp: g = 1/(1+exp(-u)); the 2x is folded in the STT
                g_ps = p_sm.tile([128, TC], F32, tag="small")
                nc.tensor.matmul(g_ps, wg_sb, xt[0:GATE_C, 0, :], start=True, stop=True)
                g_rep = wk2.tile([128, TC], F32, tag="grep")
                nc.scalar.activation(g_rep, g_ps, AF.Exp, scale=-1.0)
                nc.vector.tensor_scalar_add(g_rep, g_rep, 1.0)
                nc.vector.reciprocal(g_rep, g_rep)

                # projections + rms + rope, per source
                srcs = [("q", 0), ("q", 1), ("q", 2), ("q", 3), ("k", 0)]
                for i, (kind, h) in enumerate(srcs):
                    ps = p_q.tile([128, TC], F32, tag="q")
                    w_sb = wq_sb if kind == "q" else wk_sb
                    for e in range(NE):
                        lhs = w_sb[:, e, h * 128:(h + 1) * 128] if kind == "q" else w_sb[:, e, :]
                        nc.tensor.matmul(ps, lhs, xt[:, e, :],
                                         start=(e == 0), stop=(e == NE - 1))
                    qraw = qrp.tile([128, TC], F32, tag="qraw")
                    nc.vector.tensor_copy(qraw, ps)
                    sq = wk1.tile([128, TC], F32R, tag="sq")
                    nc.scalar.square(sq, ps)
                    ss_ps = p_sm.tile([128, TC], F32, tag="small")
                    nc.tensor.matmul(ss_ps, ones_sb, sq, start=True, stop=True)
                    rrms = wk2.tile([128, TC], F32, tag="rrms")
                    nc.scalar.activation(rrms, ss_ps, AF.Sqrt, bias=eps_sb, scale=1.0 / HD)
                    nc.vector.reciprocal(rrms, rrms)
                    qsw = wk1.tile([128, TC], F32, tag="qsw")
                    nc.sync.dma_start(out=qsw[0:64, :], in_=qraw[64:128, :])
                    nc.sync.dma_start(out=qsw[64:128, :], in_=qraw[0:64, :])
                    tA = wk1.tile([128, TC], F32, tag="tA")
                    tB = wk1.tile([128, TC], F32, tag="tB")
                    nc.vector.tensor_mul(tA, ps, c_sl)
                    nc.vector.tensor_mul(tB, qsw, s_sl)
                    nc.vector.tensor_add(tA, tA, tB)
                    dest = qyT_sb[:, h, ts:ts + TC] if kind == "q" else kT_sb[:, ts:ts + TC]
                    nc.vector.tensor_mul(dest, tA, rrms)

                # v: projection + gated ve + transpose to natural layout
                ps_v = p_q.tile([128, TC], F32, tag="q")
                for e in range(NE):
                    nc.tensor.matmul(ps_v, wv_sb[:, e, :], xt[:, e, :],
                                     start=(e == 0), stop=(e == NE - 1))
                tv = wk1.tile([128, TC], F32, tag="tA")
                nc.vector.tensor_mul(tv, ve_sl, g_rep)
                vt = wk1.tile([128, TC], F32R, tag="tB")
                nc.vector.scalar_tensor_tensor(vt, tv, 2.0, ps_v, OP.mult, OP.add)
                for tb in range(TC // 128):
                    tp_ps = p_sm.tile([128, 128], F32R, tag="small")
                    nc.tensor.transpose(tp_ps, vt[:, tb * 128:(tb + 1) * 128], eye_sb)
                    nc.vector.tensor_copy(vn_sb[:, tcix * 2 + tb, :], tp_ps)

            # ================= phase 2: windowed attention =====================
            for h in range(4):
                for qs in range(NTC):
                    q0 = qs * TC
                    kb0 = max(0, 2 * qs - 8)
                    kb1 = 2 * qs + 2
                    o_ps = p_or.tile([128, TC], F32, tag="or")
                    r_ps = p_or.tile([128, TC], F32, tag="or")
                    for j in range(kb0, kb1, 2):
                        s_ps = p_s.tile([128, 512], F32, tag="s")
                        pt = ptp.tile([128, 512], F32R, tag="pt")
                        for u in range(2):
                            kb = j + u
                            nc.tensor.matmul(s_ps[:, u * 256:(u + 1) * 256],
                                             kT_sb[:, kb * 128:(kb + 1) * 128],
                                             qyT_sb[:, h, q0:q0 + TC],
                                             start=True, stop=True)
                        nc.scalar.activation(pt, s_ps, AF.Exp, scale=float(SCALE))
                        for u in range(2):
                            kb = j + u
                            mi = None
                            if kb == 2 * qs:
                                mi = 0
                            elif kb == 2 * qs + 1:
                                mi = 1
                            elif qs >= 4 and kb == kb0:
                                mi = 2
                            elif qs >= 4 and kb == kb0 + 1:
                                mi = 3
                            ptu = pt[:, u * 256:(u + 1) * 256]
                            if mi is not None:
                                nc.vector.tensor_tensor(ptu, ptu, masks_sb[:, mi, :], OP.mult)
                            nc.tensor.matmul(o_ps, vn_sb[:, kb, :], ptu,
                                             start=(kb == kb0), stop=(kb == kb1 - 1))
                            nc.tensor.matmul(r_ps, ones_sb, ptu,
                                             start=(kb == kb0), stop=(kb == kb1 - 1))
                    rr = wk2.tile([128, TC], F32, tag="rr")
                    nc.vector.reciprocal(rr, r_ps)
                    # y^T overwrites the (now dead) q^T slice
                    nc.vector.tensor_mul(qyT_sb[:, h, q0:q0 + TC], o_ps, rr)

            # ================= phase 3: out = y @ Wo (row-parallel partial) ====
            for os_ in range(4):
                wo_sl = wop.tile([128, 4, 512], F32R, tag="wo")
                nc.sync.dma_start(
                    out=wo_sl,
                    in_=wo.rearrange("(h d) o -> d h o", d=128)[:, :, os_ * 512:(os_ + 1) * 512].bitcast(F32R),
                )
                for tt in range(T // 128):
                    pool3, tag3 = (p_s, "s") if tt % 2 == 0 else (p_or, "or")
                    po = pool3.tile([128, 512], F32, tag=tag3)
                    for h in range(4):
                        nc.tensor.matmul(po, qyT_sb[:, h, tt * 128:(tt + 1) * 128],
                                         wo_sl[:, h, :], start=(h == 0), stop=(h == 3))
                    stage = stg.tile([128, 512], F32, tag="stage")
                    if tt % 2 == 0:
                        nc.vector.tensor_copy(stage, po)
                    else:
                        nc.scalar.copy(stage, po)
                    nc.sync.dma_start(
                        out=out[tt * 128:(tt + 1) * 128, os_ * 512:(os_ + 1) * 512],
                        in_=stage)

    nc.compile()
    return nc


def _masks():
    jj = np.arange(128)[:, None]
    ii = np.arange(128)[None, :]
    tri_d = (jj <= ii).astype(np.float32)   # diag block: keep j <= i
    tri_f = (jj >= ii).astype(np.float32)   # far block: keep j >= i - WIN
    one = np.ones((128, 128), np.float32)
    zero = np.zeros((128, 128), np.float32)
    m0 = np.concatenate([tri_d, one], 1)
    m1 = np.concatenate([zero, tri_d], 1)
    m2 = np.concatenate([tri_f, zero], 1)
    m3 = np.concatenate([one, tri_f], 1)
    return np.stack([m0, m1, m2, m3])


def kernel(**inputs):
    from concourse.bass_utils import run_bass_kernel_spmd

    if "nc" not in _CACHE:
        _CACHE["nc"] = _build_program()
    nc = _CACHE["nc"]

    x = np.asarray(inputs["x"], np.float32)
    ve = np.asarray(inputs["ve"], np.float32)
    cos = np.asarray(inputs["cos"], np.float32)
    sin = np.asarray(inputs["sin"], np.float32)
    Wq = np.asarray(inputs["Wq"], np.float32)
    Wk = np.asarray(inputs["Wk"], np.float32)
    Wv = np.asarray(inputs["Wv"], np.float32)
    Wo = np.asarray(inputs["Wo"], np.float32)
    Wg = np.asarray(inputs["Wg"], np.float32)

    crep = np.ascontiguousarray(np.concatenate([cos.T, cos.T], 0))
    ssgn = np.ascontiguousarray(np.concatenate([sin.T, -sin.T], 0))
    masks = _masks()
    ones128 = np.ones((128, 128), np.float32)
    eye128 = np.eye(128, dtype=np.float32)

    in_maps = []
    for c in range(8):
        b, g = divmod(c, 4)
        in_maps.append({
            "xT": np.ascontiguousarray(x[b].T),
            "veT": np.ascontiguousarray(ve[b, :, g * HD:(g + 1) * HD].T),
            "crep": crep,
            "ssgn": ssgn,
            "wq": np.ascontiguousarray(Wq[:, g * 512:(g + 1) * 512]),
            "wk": np.ascontiguousarray(Wk[:, g * HD:(g + 1) * HD]),
            "wv": np.ascontiguousarray(Wv[:, g * HD:(g + 1) * HD]),
            "wg": np.ascontiguousarray(np.repeat(Wg[:, g:g + 1], 128, 1)),
            "wo": np.ascontiguousarray(Wo[g * 512:(g + 1) * 512, :]),
            "m_in": masks,
            "ones_in": ones128,
            "eye_in": eye128,
        })

    res = run_bass_kernel_spmd(nc, in_maps, core_ids=list(range(8)))
    parts = [res.results[c]["out"] for c in range(8)]
    out = np.stack([parts[0] + parts[1] + parts[2] + parts[3],
                    parts[4] + parts[5] + parts[6] + parts[7]])
    return out.astype(np.float32)


# revision 21
# speedup vs baseline: 1.4225x; 1.0104x over previous
"""Sliding-window causal GQA self-attention (B=2, T=2048, 16 q-heads, 4 kv-heads,
head_dim=128, window=1024) on 8 trn2 NeuronCores.

Sharding: core = (batch b, kv-group g) -> 4 query heads + 1 kv head, full T.
Wo is row-parallel; each core emits a [T, 2048] partial that the host sums per
batch (the unshard step for the row-parallel layout).

Device dataflow (all matmuls float32r, free-dim >= 256 for full PE rate):
  phase 1: qT/kT/vT projections (transposed via lhsT=W chunks, rhs=x^T chunks),
           RoPE (half-swap DMA + [c;c], [s;-s] tables), RMS-norm via ACT-square +
           all-ones-matmul replicated sum (sqrt ops paired to limit act-table
           reloads), gate sigmoid computed via Exp (shares the softmax act
           table), PE-transpose of v^T into natural V for the PV matmul.
  phase 2: S^T = K^T.T @ Q^T per 128-key block x 256-query super; ACT exp
           (scale fused); 0/1 triangle masks for window edges; PV and
           all-ones rowsum accumulated in PSUM; normalize on evacuation
           (y^T overwrites the dead q^T slice).
  phase 3: out[t, o] = sum_h yT_h^T @ Wo_h, Wo streamed per 512-col slice.
"""

import numpy as np

B, T, E = 2, 2048, 2048
NH, NKV, HD = 16, 4, 128
GATE_C = 32
WIN = 1024
EPS = 1e-6
NE = E // 128          # 16 contraction chunks
TC = 256               # phase-1 token chunk (= q-super width)
NTC = T // TC          # 8
NKB = T // 128         # 16 key blocks
SCALE = 1.0 / np.sqrt(HD)

_CACHE = {}


def _build_program():
    import concourse.bacc as bacc
    import concourse.mybir as mybir
    import concourse.tile as tile

    F32, F32R = mybir.dt.float32, mybir.dt.float32r
    AF = mybir.ActivationFunctionType
    OP = mybir.AluOpType

    nc = bacc.Bacc("TRN2", target_bir_lowering=False, debug=False, num_devices=8)

    xT = nc.dram_tensor("xT", [E, T], F32, kind="ExternalInput")
    veT = nc.dram_tensor("veT", [HD, T], F32, kind="ExternalInput")
    crep = nc.dram_tensor("crep", [128, T], F32, kind="ExternalInput")
    ssgn = nc.dram_tensor("ssgn", [128, T], F32, kind="ExternalInput")
    wq = nc.dram_tensor("wq", [E, 512], F32, kind="ExternalInput")
    wk = nc.dram_tensor("wk", [E, HD], F32, kind="ExternalInput")
    wv = nc.dram_tensor("wv", [E, HD], F32, kind="ExternalInput")
    wg = nc.dram_tensor("wg", [GATE_C, 128], F32, kind="ExternalInput")
    wo = nc.dram_tensor("wo", [512, E], F32, kind="ExternalInput")
    m_in = nc.dram_tensor("m_in", [4, 128, 512], F32, kind="ExternalInput")
    ones_in = nc.dram_tensor("ones_in", [128, 128], F32, kind="ExternalInput")
    eye_in = nc.dram_tensor("eye_in", [128, 128], F32, kind="ExternalInput")
    out = nc.dram_tensor("out", [T, E], F32, kind="ExternalOutput")

    xT_r = xT.rearrange("(e k) t -> k e t", k=128)
    wq_r = wq.rearrange("(e k) d -> k e d", k=128)
    wk_r = wk.rearrange("(e k) d -> k e d", k=128)
    wv_r = wv.rearrange("(e k) d -> k e d", k=128)

    with tile.TileContext(nc) as tc:
        from contextlib import ExitStack
        with ExitStack() as ctx:
            cst = ctx.enter_context(tc.tile_pool(name="cst", bufs=1))
            wts = ctx.enter_context(tc.tile_pool(name="wts", bufs=1))
            xtp = ctx.enter_context(tc.tile_pool(name="xtp", bufs=2))
            csl = ctx.enter_context(tc.tile_pool(name="csl", bufs=3))
            res = ctx.enter_context(tc.tile_pool(name="res", bufs=1))
            qrp = ctx.enter_context(tc.tile_pool(name="qrp", bufs=5))
            wk1 = ctx.enter_context(tc.tile_pool(name="wk1", bufs=3))
            wk2 = ctx.enter_context(tc.tile_pool(name="wk2", bufs=2))
            ptp = ctx.enter_context(tc.tile_pool(name="ptp", bufs=3))
            wop = ctx.enter_context(tc.tile_pool(name="wop", bufs=2))
            stg = ctx.enter_context(tc.tile_pool(name="stg", bufs=4))
            p_q = ctx.enter_context(tc.tile_pool(name="p_q", bufs=2, space="PSUM"))
            p_sm = ctx.enter_context(tc.tile_pool(name="p_sm", bufs=1, space="PSUM"))
            p_s = ctx.enter_context(tc.tile_pool(name="p_s", bufs=2, space="PSUM"))
            p_or = ctx.enter_context(tc.tile_pool(name="p_or", bufs=3, space="PSUM"))

            # ---- small constants ----
            masks_sb = cst.tile([128, 4, 512], F32, tag="masks")
            ones_sb = cst.tile([128, 128], F32R, tag="ones")
            eye_sb = cst.tile([128, 128], F32R, tag="eye")
            eps_sb = cst.tile([128, 1], F32, tag="eps")
            nc.sync.dma_start(out=masks_sb, in_=m_in.rearrange("m p f -> p m f"))
            nc.sync.dma_start(out=ones_sb, in_=ones_in[:].bitcast(F32R))
            nc.sync.dma_start(out=eye_sb, in_=eye_in[:].bitcast(F32R))
            nc.vector.memset(eps_sb, EPS)

            wg_sb = wts.tile([GATE_C, 128], F32R, tag="wg")
            nc.sync.dma_start(out=wg_sb, in_=wg[:].bitcast(F32R))

            # ---- chunk-0 stream DMAs FIRST so compute starts early ----
            xt0 = xtp.tile([128, NE, TC], F32R, tag="xt")
            for e4 in range(4):
                sl = slice(e4 * 4, (e4 + 1) * 4)
                nc.sync.dma_start(out=xt0[:, sl, :], in_=xT_r[:, sl, 0:TC].bitcast(F32R))
            c0 = csl.tile([128, TC], F32, tag="c_sl")
            s0 = csl.tile([128, TC], F32, tag="s_sl")
            v0 = csl.tile([HD, TC], F32, tag="ve_sl")
            nc.sync.dma_start(out=c0, in_=crep[:, 0:TC])
            nc.sync.dma_start(out=s0, in_=ssgn[:, 0:TC])
            nc.sync.dma_start(out=v0, in_=veT[:, 0:TC])

            # ---- weights, split by e-chunk groups (interleaved queues) ----
            wq_sb = wts.tile([128, NE, 512], F32R, tag="wq")
            wk_sb = wts.tile([128, NE, HD], F32R, tag="wk")
            wv_sb = wts.tile([128, NE, HD], F32R, tag="wv")
            for e4 in range(4):
                sl = slice(e4 * 4, (e4 + 1) * 4)
                nc.sync.dma_start(out=wq_sb[:, sl, :], in_=wq_r[:, sl, :].bitcast(F32R))
                nc.sync.dma_start(out=wk_sb[:, sl, :], in_=wk_r[:, sl, :].bitcast(F32R))
                nc.sync.dma_start(out=wv_sb[:, sl, :], in_=wv_r[:, sl, :].bitcast(F32R))

            # ---- persistent results (yT overwrites qT slices in phase 2) ----
            qyT_sb = res.tile([128, 4, T], F32R, tag="qyT")
            kT_sb = res.tile([128, T], F32R, tag="kT")
            vn_sb = res.tile([128, NKB, HD], F32R, tag="vn")

            # ================= phase 1 ==========================================
            for tcix in range(NTC):
                ts = tcix * TC
                if tcix == 0:
                    xt, c_sl, s_sl, ve_sl = xt0, c0, s0, v0
                else:
                    xt = xtp.tile([128, NE, TC], F32R, tag="xt")
                    nc.sync.dma_start(out=xt, in_=xT_r[:, :, ts:ts + TC].bitcast(F32R))
                    c_sl = csl.tile([128, TC], F32, tag="c_sl")
                    s_sl = csl.tile([128, TC], F32, tag="s_sl")
                    ve_sl = csl.tile([HD, TC], F32, tag="ve_sl")
                    nc.sync.dma_start(out=c_sl, in_=crep[:, ts:ts + TC])
                    nc.sync.dma_start(out=s_sl, in_=ssgn[:, ts:ts + TC])
                    nc.sync.dma_start(out=ve_sl, in_=veT[:, ts:ts + TC])

                # gate via exp: g = 1/(1+exp(-u)); the 2x is folded in the STT
                g_ps = p_sm.tile([128, TC], F32, tag="small")
                nc.tensor.matmul(g_ps, wg_sb, xt[0:GATE_C, 0, :], start=True, stop=True)
                g_rep = wk2.tile([128, TC], F32, tag="grep")
                nc.scalar.activation(g_rep, g_ps, AF.Exp, scale=-1.0)
                nc.vector.tensor_scalar_add(g_rep, g_rep, 1.0)
                nc.vector.reciprocal(g_rep, g_rep)

                # projections + rms + rope; sumsq paired per 2 srcs so each
                # Sqrt covers two sources (fewer act-table switches)
                srcs = [("q", 0), ("q", 1), ("q", 2), ("q", 3), ("k", 0)]
                chunk_qraws = []
                ss_pair = None
                rr_pair = None
                for i, (kind, h) in enumerate(srcs):
                    ps = p_q.tile([128, TC], F32, tag="q")
                    w_sb = wq_sb if kind == "q" else wk_sb
                    for e in range(NE):
                        lhs = w_sb[:, e, h * 128:(h + 1) * 128] if kind == "q" else w_sb[:, e, :]
                        nc.tensor.matmul(ps, lhs, xt[:, e, :],
                                         start=(e == 0), stop=(e == NE - 1))
                    qraw = qrp.tile([128, TC], F32, tag="qraw")
                    nc.vector.tensor_copy(qraw, ps)
                    chunk_qraws.append(qraw)
                    sq = wk1.tile([128, TC], F32R, tag="sq")
                    nc.scalar.square(sq, ps)
                    half = i % 2
                    if half == 0:
                        ss_pair = p_sm.tile([128, 512], F32, tag="small")
                        rr_pair = wk2.tile([128, 512], F32, tag="rrms")
                    nc.tensor.matmul(ss_pair[:, half * TC:(half + 1) * TC],
                                     ones_sb, sq, start=True, stop=True)
                    if half == 1 or i == 4:
                        wd = 512 if half == 1 else 256
                        nc.scalar.activation(rr_pair[:, 0:wd], ss_pair[:, 0:wd],
                                             AF.Sqrt, bias=eps_sb, scale=1.0 / HD)
                        nc.vector.reciprocal(rr_pair[:, 0:wd], rr_pair[:, 0:wd])
                        done = [i - 1, i] if half == 1 else [i]
                        for ii in done:
                            kind2, h2 = srcs[ii]
                            qraw2 = chunk_qraws[ii]
                            rrms = rr_pair[:, (ii % 2) * TC:(ii % 2 + 1) * TC]
                            qsw = wk1.tile([128, TC], F32, tag="qsw")
                            nc.sync.dma_start(out=qsw[0:64, :], in_=qraw2[64:128, :])
                            nc.sync.dma_start(out=qsw[64:128, :], in_=qraw2[0:64, :])
                            tA = wk1.tile([128, TC], F32, tag="tA")
                            tB = wk1.tile([128, TC], F32, tag="tB")
                            nc.vector.tensor_mul(tA, qraw2, c_sl)
                            nc.vector.tensor_mul(tB, qsw, s_sl)
                            nc.vector.tensor_add(tA, tA, tB)
                            dest = (qyT_sb[:, h2, ts:ts + TC] if kind2 == "q"
                                    else kT_sb[:, ts:ts + TC])
                            nc.vector.tensor_mul(dest, tA, rrms)

                # v: projection + gated ve + transpose to natural layout
                ps_v = p_q.tile([128, TC], F32, tag="q")
                for e in range(NE):
                    nc.tensor.matmul(ps_v, wv_sb[:, e, :], xt[:, e, :],
                                     start=(e == 0), stop=(e == NE - 1))
                tv = wk1.tile([128, TC], F32, tag="tA")
                nc.vector.tensor_mul(tv, ve_sl, g_rep)
                vt = wk1.tile([128, TC], F32R, tag="tB")
                nc.vector.scalar_tensor_tensor(vt, tv, 2.0, ps_v, OP.mult, OP.add)
                for tb in range(TC // 128):
                    tp_ps = p_sm.tile([128, 128], F32R, tag="small")
                    nc.tensor.transpose(tp_ps, vt[:, tb * 128:(tb + 1) * 128], eye_sb)
                    nc.vector.tensor_copy(vn_sb[:, tcix * 2 + tb, :], tp_ps)

            # ================= phase 2: windowed attention (head-paired) =======
            for hp in range(2):
                for qs in range(NTC):
                    q0 = qs * TC
                    kb0 = max(0, 2 * qs - 8)
                    kb1 = 2 * qs + 2
                    o_ps = p_or.tile([128, 512], F32, tag="or")
                    r_ps = p_or.tile([128, 512], F32, tag="or")
                    for kb in range(kb0, kb1):
                        s_ps = p_s.tile([128, 512], F32, tag="s")
                        nc.tensor.matmul(s_ps,
                                         kT_sb[:, kb * 128:(kb + 1) * 128],
                                         qyT_sb[:, 2 * hp:2 * hp + 2, q0:q0 + TC],
                                         start=True, stop=True)
                        pt = ptp.tile([128, 512], F32R, tag="pt")
                        nc.scalar.activation(pt, s_ps, AF.Exp, scale=float(SCALE))
                        mi = None
                        if kb == 2 * qs:
                            mi = 0
                        elif kb == 2 * qs + 1:
                            mi = 1
                        elif qs >= 4 and kb == kb0:
                            mi = 2
                        elif qs >= 4 and kb == kb0 + 1:
                            mi = 3
                        if mi is not None:
                            nc.vector.tensor_tensor(pt, pt, masks_sb[:, mi, :], OP.mult)
                        nc.tensor.matmul(o_ps, vn_sb[:, kb, :], pt,
                                         start=(kb == kb0), stop=(kb == kb1 - 1))
                        nc.tensor.matmul(r_ps, ones_sb, pt,
                                         start=(kb == kb0), stop=(kb == kb1 - 1))
                    rr = wk2.tile([128, 512], F32, tag="rr")
                    nc.vector.reciprocal(rr, r_ps)
                    # y^T overwrites the (now dead) q^T slices of both heads
                    nc.vector.tensor_mul(qyT_sb[:, 2 * hp:2 * hp + 2, q0:q0 + TC], o_ps, rr)

            # ================= phase 3: out = y @ Wo (row-parallel partial) ====
            for os_ in range(4):
                wo_sl = wop.tile([128, 4, 512], F32R, tag="wo")
                nc.sync.dma_start(
                    out=wo_sl,
                    in_=wo.rearrange("(h d) o -> d h o", d=128)[:, :, os_ * 512:(os_ + 1) * 512].bitcast(F32R),
                )
                for tt in range(T // 128):
                    pool3, tag3 = (p_s, "s") if tt % 2 == 0 else (p_or, "or")
                    po = pool3.tile([128, 512], F32, tag=tag3)
                    for h in range(4):
                        nc.tensor.matmul(po, qyT_sb[:, h, tt * 128:(tt + 1) * 128],
                                         wo_sl[:, h, :], start=(h == 0), stop=(h == 3))
                    stage = stg.tile([128, 512], F32, tag="stage")
                    if tt % 2 == 0:
                        nc.vector.tensor_copy(stage, po)
                    else:
                        nc.scalar.copy(stage, po)
                    nc.sync.dma_start(
                        out=out[tt * 128:(tt + 1) * 128, os_ * 512:(os_ + 1) * 512],
                        in_=stage)

    nc.compile()
    return nc


def _masks():
    jj = np.arange(128)[:, None]
    ii = np.arange(128)[None, :]
    tri_d = (jj <= ii).astype(np.float32)   # diag block: keep j <= i
    tri_f = (jj >= ii).astype(np.float32)   # far block: keep j >= i - WIN
    one = np.ones((128, 128), np.float32)
    zero = np.zeros((128, 128), np.float32)
    m0 = np.concatenate([tri_d, one], 1)
    m1 = np.concatenate([zero, tri_d], 1)
    m2 = np.concatenate([tri_f, zero], 1)
    m3 = np.concatenate([one, tri_f], 1)
    return np.ascontiguousarray(np.tile(np.stack([m0, m1, m2, m3]), (1, 1, 2)))


def kernel(**inputs):
    from concourse.bass_utils import run_bass_kernel_spmd

    if "nc" not in _CACHE:
        _CACHE["nc"] = _build_program()
    nc = _CACHE["nc"]

    x = np.asarray(inputs["x"], np.float32)
    ve = np.asarray(inputs["ve"], np.float32)
    cos = np.asarray(inputs["cos"], np.float32)
    sin = np.asarray(inputs["sin"], np.float32)
    Wq = np.asarray(inputs["Wq"], np.float32)
    Wk = np.asarray(inputs["Wk"], np.float32)
    Wv = np.asarray(inputs["Wv"], np.float32)
    Wo = np.asarray(inputs["Wo"], np.float32)
    Wg = np.asarray(inputs["Wg"], np.float32)

    crep = np.ascontiguousarray(np.concatenate([cos.T, cos.T], 0))
    ssgn = np.ascontiguousarray(np.concatenate([sin.T, -sin.T], 0))
    masks = _masks()
    ones128 = np.ones((128, 128), np.float32)
    eye128 = np.eye(128, dtype=np.float32)

    in_maps = []
    for c in range(8):
        b, g = divmod(c, 4)
        in_maps.append({
            "xT": np.ascontiguousarray(x[b].T),
            "veT": np.ascontiguousarray(ve[b, :, g * HD:(g + 1) * HD].T),
            "crep": crep,
            "ssgn": ssgn,
            "wq": np.ascontiguousarray(Wq[:, g * 512:(g + 1) * 512]),
            "wk": np.ascontiguousarray(Wk[:, g * HD:(g + 1) * HD]),
            "wv": np.ascontiguousarray(Wv[:, g * HD:(g + 1) * HD]),
            "wg": np.ascontiguousarray(np.repeat(Wg[:, g:g + 1], 128, 1)),
            "wo": np.ascontiguousarray(Wo[g * 512:(g + 1) * 512, :]),
            "m_in": masks,
            "ones_in": ones128,
            "eye_in": eye128,
        })

    res = run_bass_kernel_spmd(nc, in_maps, core_ids=list(range(8)))
    parts = [res.results[c]["out"] for c in range(8)]
    out = np.stack([parts[0] + parts[1] + parts[2] + parts[3],
                    parts[4] + parts[5] + parts[6] + parts[7]])
    return out.astype(np.float32)


# revision 26
# speedup vs baseline: 1.4752x; 1.0370x over previous
"""Sliding-window causal GQA self-attention (B=2, T=2048, 16 q-heads, 4 kv-heads,
head_dim=128, window=1024) on 8 trn2 NeuronCores.

Sharding: core = (batch b, kv-group g) -> 4 query heads + 1 kv head, full T.
Wo is row-parallel; each core emits a [T, 2048] partial that the host sums per
batch (the unshard step for the row-parallel layout).

Device dataflow (all matmuls float32r, free-dim >= 256 for full PE rate):
  phase 1: qT/kT/vT projections (transposed via lhsT=W chunks, rhs=x^T chunks),
           RoPE (half-swap DMA + [c;c], [s;-s] tables), RMS-norm via ACT-square +
           all-ones-matmul replicated sum (sqrt ops paired to limit act-table
           reloads), gate sigmoid computed via Exp (shares the softmax act
           table), PE-transpose of v^T into natural V for the PV matmul.
  phase 2: S^T = K^T.T @ Q^T per 128-key block x 256-query super; ACT exp
           (scale fused); 0/1 triangle masks for window edges; PV and
           all-ones rowsum accumulated in PSUM; normalize on evacuation
           (y^T overwrites the dead q^T slice).
  phase 3: out[t, o] = sum_h yT_h^T @ Wo_h, Wo streamed per 512-col slice.
"""

import numpy as np

B, T, E = 2, 2048, 2048
NH, NKV, HD = 16, 4, 128
GATE_C = 32
WIN = 1024
EPS = 1e-6
NE = E // 128          # 16 contraction chunks
TC = 256               # phase-1 token chunk (= q-super width)
NTC = T // TC          # 8
NKB = T // 128         # 16 key blocks
SCALE = 1.0 / np.sqrt(HD)

_CACHE = {}


def _build_program():
    import concourse.bacc as bacc
    import concourse.mybir as mybir
    import concourse.tile as tile

    F32, F32R = mybir.dt.float32, mybir.dt.float32r
    AF = mybir.ActivationFunctionType
    OP = mybir.AluOpType

    nc = bacc.Bacc("TRN2", target_bir_lowering=False, debug=False, num_devices=8)

    xT = nc.dram_tensor("xT", [E, T], F32, kind="ExternalInput")
    veT = nc.dram_tensor("veT", [HD, T], F32, kind="ExternalInput")
    crep = nc.dram_tensor("crep", [128, T], F32, kind="ExternalInput")
    ssgn = nc.dram_tensor("ssgn", [128, T], F32, kind="ExternalInput")
    wq = nc.dram_tensor("wq", [E, 512], F32, kind="ExternalInput")
    wk = nc.dram_tensor("wk", [E, HD], F32, kind="ExternalInput")
    wv = nc.dram_tensor("wv", [E, HD], F32, kind="ExternalInput")
    wg = nc.dram_tensor("wg", [GATE_C, 128], F32, kind="ExternalInput")
    wo = nc.dram_tensor("wo", [512, E], F32, kind="ExternalInput")
    m_in = nc.dram_tensor("m_in", [4, 128, 512], F32, kind="ExternalInput")
    ones_in = nc.dram_tensor("ones_in", [128, 128], F32, kind="ExternalInput")
    eye_in = nc.dram_tensor("eye_in", [128, 128], F32, kind="ExternalInput")
    out = nc.dram_tensor("out", [T, E], F32, kind="ExternalOutput")

    xT_r = xT.rearrange("(e k) t -> k e t", k=128)
    wq_r = wq.rearrange("(e k) d -> k e d", k=128)
    wk_r = wk.rearrange("(e k) d -> k e d", k=128)
    wv_r = wv.rearrange("(e k) d -> k e d", k=128)

    with tile.TileContext(nc) as tc:
        from contextlib import ExitStack
        with ExitStack() as ctx:
            cst = ctx.enter_context(tc.tile_pool(name="cst", bufs=1))
            wts = ctx.enter_context(tc.tile_pool(name="wts", bufs=1))
            xtp = ctx.enter_context(tc.tile_pool(name="xtp", bufs=2))
            csl = ctx.enter_context(tc.tile_pool(name="csl", bufs=3))
            res = ctx.enter_context(tc.tile_pool(name="res", bufs=1))
            qrp = ctx.enter_context(tc.tile_pool(name="qrp", bufs=5))
            wk1 = ctx.enter_context(tc.tile_pool(name="wk1", bufs=3))
            wk2 = ctx.enter_context(tc.tile_pool(name="wk2", bufs=2))
            ptp = ctx.enter_context(tc.tile_pool(name="ptp", bufs=4))
            wop = ctx.enter_context(tc.tile_pool(name="wop", bufs=2))
            stg = ctx.enter_context(tc.tile_pool(name="stg", bufs=4))
            p_q = ctx.enter_context(tc.tile_pool(name="p_q", bufs=2, space="PSUM"))
            p_sm = ctx.enter_context(tc.tile_pool(name="p_sm", bufs=1, space="PSUM"))
            p_s = ctx.enter_context(tc.tile_pool(name="p_s", bufs=3, space="PSUM"))
            p_or = ctx.enter_context(tc.tile_pool(name="p_or", bufs=2, space="PSUM"))

            # ---- small constants ----
            masks_sb = cst.tile([128, 4, 512], F32, tag="masks")
            ones_sb = cst.tile([128, 128], F32R, tag="ones")
            eye_sb = cst.tile([128, 128], F32R, tag="eye")
            eps_sb = cst.tile([128, 1], F32, tag="eps")
            nc.sync.dma_start(out=masks_sb, in_=m_in.rearrange("m p f -> p m f"))
            nc.sync.dma_start(out=ones_sb, in_=ones_in[:].bitcast(F32R))
            nc.sync.dma_start(out=eye_sb, in_=eye_in[:].bitcast(F32R))
            nc.vector.memset(eps_sb, EPS)

            wg_sb = wts.tile([GATE_C, 128], F32R, tag="wg")
            nc.sync.dma_start(out=wg_sb, in_=wg[:].bitcast(F32R))

            # ---- chunk-0 stream DMAs FIRST so compute starts early ----
            xt0 = xtp.tile([128, NE, TC], F32R, tag="xt")
            for e4 in range(4):
                sl = slice(e4 * 4, (e4 + 1) * 4)
                nc.sync.dma_start(out=xt0[:, sl, :], in_=xT_r[:, sl, 0:TC].bitcast(F32R))
            c0 = csl.tile([128, TC], F32, tag="c_sl")
            s0 = csl.tile([128, TC], F32, tag="s_sl")
            v0 = csl.tile([HD, TC], F32, tag="ve_sl")
            nc.sync.dma_start(out=c0, in_=crep[:, 0:TC])
            nc.sync.dma_start(out=s0, in_=ssgn[:, 0:TC])
            nc.sync.dma_start(out=v0, in_=veT[:, 0:TC])

            # ---- weights, split by e-chunk groups (interleaved queues) ----
            wq_sb = wts.tile([128, NE, 512], F32R, tag="wq")
            wk_sb = wts.tile([128, NE, HD], F32R, tag="wk")
            wv_sb = wts.tile([128, NE, HD], F32R, tag="wv")
            for e4 in range(4):
                sl = slice(e4 * 4, (e4 + 1) * 4)
                nc.sync.dma_start(out=wq_sb[:, sl, :], in_=wq_r[:, sl, :].bitcast(F32R))
                nc.sync.dma_start(out=wk_sb[:, sl, :], in_=wk_r[:, sl, :].bitcast(F32R))
                nc.sync.dma_start(out=wv_sb[:, sl, :], in_=wv_r[:, sl, :].bitcast(F32R))

            # ---- persistent results (yT overwrites qT slices in phase 2) ----
            qyT_sb = res.tile([128, 4, T], F32R, tag="qyT")
            kT_sb = res.tile([128, T], F32R, tag="kT")
            vn_sb = res.tile([128, NKB, HD], F32R, tag="vn")

            # ================= phase 1 ==========================================
            for tcix in range(NTC):
                ts = tcix * TC
                if tcix == 0:
                    xt, c_sl, s_sl, ve_sl = xt0, c0, s0, v0
                else:
                    xt = xtp.tile([128, NE, TC], F32R, tag="xt")
                    nc.sync.dma_start(out=xt, in_=xT_r[:, :, ts:ts + TC].bitcast(F32R))
                    c_sl = csl.tile([128, TC], F32, tag="c_sl")
                    s_sl = csl.tile([128, TC], F32, tag="s_sl")
                    ve_sl = csl.tile([HD, TC], F32, tag="ve_sl")
                    nc.sync.dma_start(out=c_sl, in_=crep[:, ts:ts + TC])
                    nc.sync.dma_start(out=s_sl, in_=ssgn[:, ts:ts + TC])
                    nc.sync.dma_start(out=ve_sl, in_=veT[:, ts:ts + TC])

                # gate via exp: g = 1/(1+exp(-u)); the 2x is folded in the STT
                g_ps = p_sm.tile([128, TC], F32, tag="small")
                nc.tensor.matmul(g_ps, wg_sb, xt[0:GATE_C, 0, :], start=True, stop=True)
                g_rep = wk2.tile([128, TC], F32, tag="grep")
                nc.scalar.activation(g_rep, g_ps, AF.Exp, scale=-1.0)
                nc.vector.tensor_scalar_add(g_rep, g_rep, 1.0)
                nc.vector.reciprocal(g_rep, g_rep)

                # projections + rms + rope; sumsq paired per 2 srcs so each
                # Sqrt covers two sources (fewer act-table switches)
                srcs = [("q", 0), ("q", 1), ("q", 2), ("q", 3), ("k", 0)]
                chunk_qraws = []
                ss_pair = None
                rr_pair = None
                for i, (kind, h) in enumerate(srcs):
                    ps = p_q.tile([128, TC], F32, tag="q")
                    w_sb = wq_sb if kind == "q" else wk_sb
                    for e in range(NE):
                        lhs = w_sb[:, e, h * 128:(h + 1) * 128] if kind == "q" else w_sb[:, e, :]
                        nc.tensor.matmul(ps, lhs, xt[:, e, :],
                                         start=(e == 0), stop=(e == NE - 1))
                    qraw = qrp.tile([128, TC], F32, tag="qraw")
                    nc.vector.tensor_copy(qraw, ps)
                    chunk_qraws.append(qraw)
                    sq = wk1.tile([128, TC], F32R, tag="sq")
                    nc.scalar.square(sq, ps)
                    half = i % 2
                    if half == 0:
                        ss_pair = p_sm.tile([128, 512], F32, tag="small")
                        rr_pair = wk2.tile([128, 512], F32, tag="rrms")
                    nc.tensor.matmul(ss_pair[:, half * TC:(half + 1) * TC],
                                     ones_sb, sq, start=True, stop=True)
                    if half == 1 or i == 4:
                        wd = 512 if half == 1 else 256
                        nc.scalar.activation(rr_pair[:, 0:wd], ss_pair[:, 0:wd],
                                             AF.Sqrt, bias=eps_sb, scale=1.0 / HD)
                        nc.vector.reciprocal(rr_pair[:, 0:wd], rr_pair[:, 0:wd])
                        done = [i - 1, i] if half == 1 else [i]
                        for ii in done:
                            kind2, h2 = srcs[ii]
                            qraw2 = chunk_qraws[ii]
                            rrms = rr_pair[:, (ii % 2) * TC:(ii % 2 + 1) * TC]
                            qsw = wk1.tile([128, TC], F32, tag="qsw")
                            nc.sync.dma_start(out=qsw[0:64, :], in_=qraw2[64:128, :])
                            nc.sync.dma_start(out=qsw[64:128, :], in_=qraw2[0:64, :])
                            tA = wk1.tile([128, TC], F32, tag="tA")
                            tB = wk1.tile([128, TC], F32, tag="tB")
                            nc.vector.tensor_mul(tA, qraw2, c_sl)
                            nc.vector.tensor_mul(tB, qsw, s_sl)
                            nc.vector.tensor_add(tA, tA, tB)
                            dest = (qyT_sb[:, h2, ts:ts + TC] if kind2 == "q"
                                    else kT_sb[:, ts:ts + TC])
                            nc.vector.tensor_mul(dest, tA, rrms)

                # v: projection + gated ve + transpose to natural layout
                ps_v = p_q.tile([128, TC], F32, tag="q")
                for e in range(NE):
                    nc.tensor.matmul(ps_v, wv_sb[:, e, :], xt[:, e, :],
                                     start=(e == 0), stop=(e == NE - 1))
                tv = wk1.tile([128, TC], F32, tag="tA")
                nc.vector.tensor_mul(tv, ve_sl, g_rep)
                vt = wk1.tile([128, TC], F32R, tag="tB")
                nc.vector.scalar_tensor_tensor(vt, tv, 2.0, ps_v, OP.mult, OP.add)
                for tb in range(TC // 128):
                    tp_ps = p_sm.tile([128, 128], F32R, tag="small")
                    nc.tensor.transpose(tp_ps, vt[:, tb * 128:(tb + 1) * 128], eye_sb)
                    nc.vector.tensor_copy(vn_sb[:, tcix * 2 + tb, :], tp_ps)

            # ================= phase 2: windowed attention (head-paired) =======
            for hp in range(2):
                for qs in range(NTC):
                    q0 = qs * TC
                    kb0 = max(0, 2 * qs - 8)
                    kb1 = 2 * qs + 2
                    o_ps = p_or.tile([128, 512], F32, tag="or")
                    r_ps = p_or.tile([128, 512], F32, tag="or")
                    for kb in range(kb0, kb1):
                        s_ps = p_s.tile([128, 512], F32, tag="s")
                        nc.tensor.matmul(s_ps,
                                         kT_sb[:, kb * 128:(kb + 1) * 128],
                                         qyT_sb[:, 2 * hp:2 * hp + 2, q0:q0 + TC],
                                         start=True, stop=True)
                        pt = ptp.tile([128, 512], F32R, tag="pt")
                        nc.scalar.activation(pt, s_ps, AF.Exp, scale=float(SCALE))
                        mi = None
                        if kb == 2 * qs:
                            mi = 0
                        elif kb == 2 * qs + 1:
                            mi = 1
                        elif qs >= 4 and kb == kb0:
                            mi = 2
                        elif qs >= 4 and kb == kb0 + 1:
                            mi = 3
                        if mi is not None:
                            nc.vector.tensor_tensor(pt, pt, masks_sb[:, mi, :], OP.mult)
                        nc.tensor.matmul(o_ps, vn_sb[:, kb, :], pt,
                                         start=(kb == kb0), stop=(kb == kb1 - 1))
                        nc.tensor.matmul(r_ps, ones_sb, pt,
                                         start=(kb == kb0), stop=(kb == kb1 - 1))
                    rr = wk2.tile([128, 512], F32, tag="rr")
                    nc.vector.reciprocal(rr, r_ps)
                    # y^T overwrites the (now dead) q^T slices of both heads
                    nc.vector.tensor_mul(qyT_sb[:, 2 * hp:2 * hp + 2, q0:q0 + TC], o_ps, rr)

            # ================= phase 3: out = y @ Wo (row-parallel partial) ====
            for os_ in range(4):
                wo_sl = wop.tile([128, 4, 512], F32R, tag="wo")
                nc.sync.dma_start(
                    out=wo_sl,
                    in_=wo.rearrange("(h d) o -> d h o", d=128)[:, :, os_ * 512:(os_ + 1) * 512].bitcast(F32R),
                )
                for tt in range(T // 128):
                    pool3, tag3 = (p_s, "s") if tt % 2 == 0 else (p_or, "or")
                    po = pool3.tile([128, 512], F32, tag=tag3)
                    for h in range(4):
                        nc.tensor.matmul(po, qyT_sb[:, h, tt * 128:(tt + 1) * 128],
                                         wo_sl[:, h, :], start=(h == 0), stop=(h == 3))
                    stage = stg.tile([128, 512], F32, tag="stage")
                    if tt % 2 == 0:
                        nc.vector.tensor_copy(stage, po)
                    else:
                        nc.scalar.copy(stage, po)
                    nc.sync.dma_start(
                        out=out[tt * 128:(tt + 1) * 128, os_ * 512:(os_ + 1) * 512],
                        in_=stage)

    nc.compile()
    return nc


def _masks():
    jj = np.arange(128)[:, None]
    ii = np.arange(128)[None, :]
    tri_d = (jj <= ii).astype(np.float32)   # diag block: keep j <= i
    tri_f = (jj >= ii).astype(np.float32)   # far block: keep j >= i - WIN
    one = np.ones((128, 128), np.float32)
    zero = np.zeros((128, 128), np.float32)
    m0 = np.concatenate([tri_d, one], 1)
    m1 = np.concatenate([zero, tri_d], 1)
    m2 = np.concatenate([tri_f, zero], 1)
    m3 = np.concatenate([one, tri_f], 1)
    return np.ascontiguousarray(np.tile(np.stack([m0, m1, m2, m3]), (1, 1, 2)))


def kernel(**inputs):
    from concourse.bass_utils import run_bass_kernel_spmd

    if "nc" not in _CACHE:
        _CACHE["nc"] = _build_program()
    nc = _CACHE["nc"]

    x = np.asarray(inputs["x"], np.float32)
    ve = np.asarray(inputs["ve"], np.float32)
    cos = np.asarray(inputs["cos"], np.float32)
    sin = np.asarray(inputs["sin"], np.float32)
    Wq = np.asarray(inputs["Wq"], np.float32)
    Wk = np.asarray(inputs["Wk"], np.float32)
    Wv = np.asarray(inputs["Wv"], np.float32)
    Wo = np.asarray(inputs["Wo"], np.float32)
    Wg = np.asarray(inputs["Wg"], np.float32)

    crep = np.ascontiguousarray(np.concatenate([cos.T, cos.T], 0))
    ssgn = np.ascontiguousarray(np.concatenate([sin.T, -sin.T], 0))
    masks = _masks()
    ones128 = np.ones((128, 128), np.float32)
    eye128 = np.eye(128, dtype=np.float32)

    in_maps = []
    for c in range(8):
        b, g = divmod(c, 4)
        in_maps.append({
            "xT": np.ascontiguousarray(x[b].T),
            "veT": np.ascontiguousarray(ve[b, :, g * HD:(g + 1) * HD].T),
            "crep": crep,
            "ssgn": ssgn,
            "wq": np.ascontiguousarray(Wq[:, g * 512:(g + 1) * 512]),
            "wk": np.ascontiguousarray(Wk[:, g * HD:(g + 1) * HD]),
            "wv": np.ascontiguousarray(Wv[:, g * HD:(g + 1) * HD]),
            "wg": np.ascontiguousarray(np.repeat(Wg[:, g:g + 1], 128, 1)),
            "wo": np.ascontiguousarray(Wo[g * 512:(g + 1) * 512, :]),
            "m_in": masks,
            "ones_in": ones128,
            "eye_in": eye128,
        })

    res = run_bass_kernel_spmd(nc, in_maps, core_ids=list(range(8)))
    parts = [res.results[c]["out"] for c in range(8)]
    out = np.stack([parts[0] + parts[1] + parts[2] + parts[3],
                    parts[4] + parts[5] + parts[6] + parts[7]])
    return out.astype(np.float32)


# revision 29
# speedup vs baseline: 1.4857x; 1.0072x over previous
"""Sliding-window causal GQA self-attention (B=2, T=2048, 16 q-heads, 4 kv-heads,
head_dim=128, window=1024) on 8 trn2 NeuronCores.

Sharding: core = (batch b, kv-group g) -> 4 query heads + 1 kv head, full T.
Wo is row-parallel; each core emits a [T, 2048] partial that the host sums per
batch (the unshard step for the row-parallel layout).

Device dataflow (all matmuls float32r, free-dim >= 256 for full PE rate):
  phase 1: qT/kT/vT projections (transposed via lhsT=W chunks, rhs=x^T chunks),
           RoPE (half-swap DMA + [c;c], [s;-s] tables), RMS-norm via ACT-square +
           all-ones-matmul replicated sum (sqrt ops paired to limit act-table
           reloads), gate sigmoid computed via Exp (shares the softmax act
           table), PE-transpose of v^T into natural V for the PV matmul.
  phase 2: S^T = K^T.T @ Q^T per 128-key block x 256-query super; ACT exp
           (scale fused); 0/1 triangle masks for window edges; PV and
           all-ones rowsum accumulated in PSUM; normalize on evacuation
           (y^T overwrites the dead q^T slice).
  phase 3: out[t, o] = sum_h yT_h^T @ Wo_h, Wo streamed per 512-col slice.
"""

import numpy as np

B, T, E = 2, 2048, 2048
NH, NKV, HD = 16, 4, 128
GATE_C = 32
WIN = 1024
EPS = 1e-6
NE = E // 128          # 16 contraction chunks
TC = 256               # phase-1 token chunk (= q-super width)
NTC = T // TC          # 8
NKB = T // 128         # 16 key blocks
SCALE = 1.0 / np.sqrt(HD)

_CACHE = {}


def _build_program():
    import concourse.bacc as bacc
    import concourse.mybir as mybir
    import concourse.tile as tile

    F32, F32R = mybir.dt.float32, mybir.dt.float32r
    AF = mybir.ActivationFunctionType
    OP = mybir.AluOpType

    nc = bacc.Bacc("TRN2", target_bir_lowering=False, debug=False, num_devices=8)

    xT = nc.dram_tensor("xT", [E, T], F32, kind="ExternalInput")
    veT = nc.dram_tensor("veT", [HD, T], F32, kind="ExternalInput")
    crep = nc.dram_tensor("crep", [128, T], F32, kind="ExternalInput")
    ssgn = nc.dram_tensor("ssgn", [128, T], F32, kind="ExternalInput")
    wq = nc.dram_tensor("wq", [E, 512], F32, kind="ExternalInput")
    wk = nc.dram_tensor("wk", [E, HD], F32, kind="ExternalInput")
    wv = nc.dram_tensor("wv", [E, HD], F32, kind="ExternalInput")
    wg = nc.dram_tensor("wg", [GATE_C, 128], F32, kind="ExternalInput")
    wo = nc.dram_tensor("wo", [512, E], F32, kind="ExternalInput")
    m_in = nc.dram_tensor("m_in", [4, 128, 512], F32, kind="ExternalInput")
    ones_in = nc.dram_tensor("ones_in", [128, 128], F32, kind="ExternalInput")
    eye_in = nc.dram_tensor("eye_in", [128, 128], F32, kind="ExternalInput")
    out = nc.dram_tensor("out", [T, E], F32, kind="ExternalOutput")

    xT_r = xT.rearrange("(e k) t -> k e t", k=128)
    wq_r = wq.rearrange("(e k) d -> k e d", k=128)
    wk_r = wk.rearrange("(e k) d -> k e d", k=128)
    wv_r = wv.rearrange("(e k) d -> k e d", k=128)

    with tile.TileContext(nc) as tc:
        from contextlib import ExitStack
        with ExitStack() as ctx:
            cst = ctx.enter_context(tc.tile_pool(name="cst", bufs=1))
            wts = ctx.enter_context(tc.tile_pool(name="wts", bufs=1))
            xtp = ctx.enter_context(tc.tile_pool(name="xtp", bufs=2))
            csl = ctx.enter_context(tc.tile_pool(name="csl", bufs=2))
            res = ctx.enter_context(tc.tile_pool(name="res", bufs=1))
            qrp = ctx.enter_context(tc.tile_pool(name="qrp", bufs=5))
            wk1 = ctx.enter_context(tc.tile_pool(name="wk1", bufs=3))
            wk2 = ctx.enter_context(tc.tile_pool(name="wk2", bufs=2))
            ptp = ctx.enter_context(tc.tile_pool(name="ptp", bufs=4))
            wop = ctx.enter_context(tc.tile_pool(name="wop", bufs=2))
            stg = ctx.enter_context(tc.tile_pool(name="stg", bufs=4))
            p_q = ctx.enter_context(tc.tile_pool(name="p_q", bufs=2, space="PSUM"))
            p_sm = ctx.enter_context(tc.tile_pool(name="p_sm", bufs=1, space="PSUM"))
            p_s = ctx.enter_context(tc.tile_pool(name="p_s", bufs=3, space="PSUM"))
            p_or = ctx.enter_context(tc.tile_pool(name="p_or", bufs=2, space="PSUM"))

            # ---- small constants ----
            masks_sb = cst.tile([128, 4, 512], F32, tag="masks")
            ones_sb = cst.tile([128, 128], F32R, tag="ones")
            eye_sb = cst.tile([128, 128], F32R, tag="eye")
            eps_sb = cst.tile([128, 1], F32, tag="eps")
            nc.sync.dma_start(out=masks_sb, in_=m_in.rearrange("m p f -> p m f"))
            nc.sync.dma_start(out=ones_sb, in_=ones_in[:].bitcast(F32R))
            nc.sync.dma_start(out=eye_sb, in_=eye_in[:].bitcast(F32R))
            nc.vector.memset(eps_sb, EPS)

            wg_sb = wts.tile([GATE_C, 128], F32R, tag="wg")
            nc.sync.dma_start(out=wg_sb, in_=wg[:].bitcast(F32R))

            # ---- chunk-0 stream DMAs FIRST so compute starts early ----
            xt0 = xtp.tile([128, NE, TC], F32R, tag="xt")
            for e4 in range(4):
                sl = slice(e4 * 4, (e4 + 1) * 4)
                nc.sync.dma_start(out=xt0[:, sl, :], in_=xT_r[:, sl, 0:TC].bitcast(F32R))
            c0 = csl.tile([128, TC], F32, tag="c_sl")
            s0 = csl.tile([128, TC], F32, tag="s_sl")
            v0 = csl.tile([HD, TC], F32, tag="ve_sl")
            nc.sync.dma_start(out=c0, in_=crep[:, 0:TC])
            nc.sync.dma_start(out=s0, in_=ssgn[:, 0:TC])
            nc.sync.dma_start(out=v0, in_=veT[:, 0:TC])

            # ---- weights, split by e-chunk groups (interleaved queues) ----
            wq_sb = wts.tile([128, NE, 512], F32R, tag="wq")
            wk_sb = wts.tile([128, NE, HD], F32R, tag="wk")
            wv_sb = wts.tile([128, NE, HD], F32R, tag="wv")
            for e4 in range(4):
                sl = slice(e4 * 4, (e4 + 1) * 4)
                nc.sync.dma_start(out=wq_sb[:, sl, :], in_=wq_r[:, sl, :].bitcast(F32R))
                nc.sync.dma_start(out=wk_sb[:, sl, :], in_=wk_r[:, sl, :].bitcast(F32R))
                nc.sync.dma_start(out=wv_sb[:, sl, :], in_=wv_r[:, sl, :].bitcast(F32R))

            # ---- persistent results (yT overwrites qT slices in phase 2) ----
            qyT_sb = res.tile([128, 4, T], F32R, tag="qyT")
            kT_sb = res.tile([128, T], F32R, tag="kT")
            vn_sb = res.tile([128, NKB, HD], F32R, tag="vn")

            # ================= phase 1 ==========================================
            for tcix in range(NTC):
                ts = tcix * TC
                if tcix == 0:
                    xt, c_sl, s_sl, ve_sl = xt0, c0, s0, v0
                else:
                    xt = xtp.tile([128, NE, TC], F32R, tag="xt")
                    nc.sync.dma_start(out=xt, in_=xT_r[:, :, ts:ts + TC].bitcast(F32R))
                    c_sl = csl.tile([128, TC], F32, tag="c_sl")
                    s_sl = csl.tile([128, TC], F32, tag="s_sl")
                    ve_sl = csl.tile([HD, TC], F32, tag="ve_sl")
                    nc.sync.dma_start(out=c_sl, in_=crep[:, ts:ts + TC])
                    nc.sync.dma_start(out=s_sl, in_=ssgn[:, ts:ts + TC])
                    nc.sync.dma_start(out=ve_sl, in_=veT[:, ts:ts + TC])

                # gate via exp: g = 1/(1+exp(-u)); the 2x is folded in the STT
                g_ps = p_sm.tile([128, TC], F32, tag="small")
                nc.tensor.matmul(g_ps, wg_sb, xt[0:GATE_C, 0, :], start=True, stop=True)
                g_rep = wk2.tile([128, TC], F32, tag="grep")
                nc.scalar.activation(g_rep, g_ps, AF.Exp, scale=-1.0)
                nc.vector.tensor_scalar_add(g_rep, g_rep, 1.0)
                nc.vector.reciprocal(g_rep, g_rep)

                # projections + rms + rope; sumsq paired per 2 srcs so each
                # Sqrt covers two sources (fewer act-table switches)
                srcs = [("q", 0), ("q", 1), ("q", 2), ("q", 3), ("k", 0)]
                chunk_qraws = []
                ss_pair = None
                rr_pair = None
                for i, (kind, h) in enumerate(srcs):
                    ps = p_q.tile([128, TC], F32, tag="q")
                    w_sb = wq_sb if kind == "q" else wk_sb
                    for e in range(NE):
                        lhs = w_sb[:, e, h * 128:(h + 1) * 128] if kind == "q" else w_sb[:, e, :]
                        nc.tensor.matmul(ps, lhs, xt[:, e, :],
                                         start=(e == 0), stop=(e == NE - 1))
                    qraw = qrp.tile([128, TC], F32, tag="qraw")
                    nc.vector.tensor_copy(qraw, ps)
                    chunk_qraws.append(qraw)
                    sq = wk1.tile([128, TC], F32R, tag="sq")
                    nc.scalar.square(sq, ps)
                    half = i % 2
                    if half == 0:
                        ss_pair = p_sm.tile([128, 512], F32, tag="small")
                        rr_pair = wk2.tile([128, 512], F32, tag="rrms")
                    nc.tensor.matmul(ss_pair[:, half * TC:(half + 1) * TC],
                                     ones_sb, sq, start=True, stop=True)
                    if half == 1 or i == 4:
                        wd = 512 if half == 1 else 256
                        nc.scalar.activation(rr_pair[:, 0:wd], ss_pair[:, 0:wd],
                                             AF.Sqrt, bias=eps_sb, scale=1.0 / HD)
                        nc.vector.reciprocal(rr_pair[:, 0:wd], rr_pair[:, 0:wd])
                        done = [i - 1, i] if half == 1 else [i]
                        for ii in done:
                            kind2, h2 = srcs[ii]
                            qraw2 = chunk_qraws[ii]
                            rrms = rr_pair[:, (ii % 2) * TC:(ii % 2 + 1) * TC]
                            qsw = wk1.tile([128, TC], F32, tag="qsw")
                            nc.sync.dma_start(out=qsw[0:64, :], in_=qraw2[64:128, :])
                            nc.sync.dma_start(out=qsw[64:128, :], in_=qraw2[0:64, :])
                            tA = wk1.tile([128, TC], F32, tag="tA")
                            tB = wk1.tile([128, TC], F32, tag="tB")
                            nc.vector.tensor_mul(tA, qraw2, c_sl)
                            nc.vector.tensor_mul(tB, qsw, s_sl)
                            nc.vector.tensor_add(tA, tA, tB)
                            dest = (qyT_sb[:, h2, ts:ts + TC] if kind2 == "q"
                                    else kT_sb[:, ts:ts + TC])
                            nc.vector.tensor_mul(dest, tA, rrms)

                # v: projection + gated ve + transpose to natural layout
                ps_v = p_q.tile([128, TC], F32, tag="q")
                for e in range(NE):
                    nc.tensor.matmul(ps_v, wv_sb[:, e, :], xt[:, e, :],
                                     start=(e == 0), stop=(e == NE - 1))
                tv = wk1.tile([128, TC], F32, tag="tA")
                nc.vector.tensor_mul(tv, ve_sl, g_rep)
                vt = wk1.tile([128, TC], F32R, tag="tB")
                nc.vector.scalar_tensor_tensor(vt, tv, 2.0, ps_v, OP.mult, OP.add)
                for tb in range(TC // 128):
                    tp_ps = p_sm.tile([128, 128], F32R, tag="small")
                    nc.tensor.transpose(tp_ps, vt[:, tb * 128:(tb + 1) * 128], eye_sb)
                    nc.vector.tensor_copy(vn_sb[:, tcix * 2 + tb, :], tp_ps)

            # ================= phase 2: windowed attention (head-paired) =======
            for hp in range(2):
                h2 = slice(2 * hp, 2 * hp + 2)
                for qs in range(NTC):
                    q0 = qs * TC
                    kb0 = max(0, 2 * qs - 8)
                    kb1 = 2 * qs + 2
                    o_ps = p_or.tile([128, 512], F32, tag="or")
                    r_ps = p_or.tile([128, 512], F32, tag="or")
                    for kb in range(kb0, kb1):
                        s_ps = p_s.tile([128, 512], F32, tag="s")
                        nc.tensor.matmul(s_ps,
                                         kT_sb[:, kb * 128:(kb + 1) * 128],
                                         qyT_sb[:, h2, q0:q0 + TC],
                                         start=True, stop=True)
                        pt = ptp.tile([128, 512], F32R, tag="pt")
                        nc.scalar.activation(pt, s_ps, AF.Exp, scale=float(SCALE))
                        mi = None
                        if kb == 2 * qs:
                            mi = 0
                        elif kb == 2 * qs + 1:
                            mi = 1
                        elif qs >= 4 and kb == kb0:
                            mi = 2
                        elif qs >= 4 and kb == kb0 + 1:
                            mi = 3
                        if mi is not None:
                            nc.vector.tensor_tensor(pt, pt, masks_sb[:, mi, :], OP.mult)
                        nc.tensor.matmul(o_ps, vn_sb[:, kb, :], pt,
                                         start=(kb == kb0), stop=(kb == kb1 - 1))
                        nc.tensor.matmul(r_ps, ones_sb, pt,
                                         start=(kb == kb0), stop=(kb == kb1 - 1))
                    rr = wk2.tile([128, 512], F32, tag="rr")
                    nc.vector.reciprocal(rr, r_ps)
                    # y^T overwrites the (now dead) q^T slices of both heads
                    nc.vector.tensor_mul(qyT_sb[:, h2, q0:q0 + TC], o_ps, rr)

            # ================= phase 3: out = y @ Wo (row-parallel partial) ====
            for os_ in range(4):
                wo_sl = wop.tile([128, 4, 512], F32R, tag="wo")
                nc.sync.dma_start(
                    out=wo_sl,
                    in_=wo.rearrange("(h d) o -> d h o", d=128)[:, :, os_ * 512:(os_ + 1) * 512].bitcast(F32R),
                )
                for tt in range(T // 128):
                    pool3, tag3 = (p_s, "s") if tt % 2 == 0 else (p_or, "or")
                    po = pool3.tile([128, 512], F32, tag=tag3)
                    for h in range(4):
                        nc.tensor.matmul(po, qyT_sb[:, h, tt * 128:(tt + 1) * 128],
                                         wo_sl[:, h, :], start=(h == 0), stop=(h == 3))
                    stage = stg.tile([128, 512], F32, tag="stage")
                    if tt % 2 == 0:
                        nc.vector.tensor_copy(stage, po)
                    else:
                        nc.scalar.copy(stage, po)
                    nc.sync.dma_start(
                        out=out[tt * 128:(tt + 1) * 128, os_ * 512:(os_ + 1) * 512],
                        in_=stage)

    nc.compile()
    return nc


def _masks():
    jj = np.arange(128)[:, None]
    ii = np.arange(128)[None, :]
    tri_d = (jj <= ii).astype(np.float32)   # diag block: keep j <= i
    tri_f = (jj >= ii).astype(np.float32)   # far block: keep j >= i - WIN
    one = np.ones((128, 128), np.float32)
    zero = np.zeros((128, 128), np.float32)
    m0 = np.concatenate([tri_d, one], 1)
    m1 = np.concatenate([zero, tri_d], 1)
    m2 = np.concatenate([tri_f, zero], 1)
    m3 = np.concatenate([one, tri_f], 1)
    return np.ascontiguousarray(np.tile(np.stack([m0, m1, m2, m3]), (1, 1, 2)))


def kernel(**inputs):
    from concourse.bass_utils import run_bass_kernel_spmd

    if "nc" not in _CACHE:
        _CACHE["nc"] = _build_program()
    nc = _CACHE["nc"]

    x = np.asarray(inputs["x"], np.float32)
    ve = np.asarray(inputs["ve"], np.float32)
    cos = np.asarray(inputs["cos"], np.float32)
    sin = np.asarray(inputs["sin"], np.float32)
    Wq = np.asarray(inputs["Wq"], np.float32)
    Wk = np.asarray(inputs["Wk"], np.float32)
    Wv = np.asarray(inputs["Wv"], np.float32)
    Wo = np.asarray(inputs["Wo"], np.float32)
    Wg = np.asarray(inputs["Wg"], np.float32)

    crep = np.ascontiguousarray(np.concatenate([cos.T, cos.T], 0))
    ssgn = np.ascontiguousarray(np.concatenate([sin.T, -sin.T], 0))
    masks = _masks()
    ones128 = np.ones((128, 128), np.float32)
    eye128 = np.eye(128, dtype=np.float32)

    in_maps = []
    for c in range(8):
        b, g = divmod(c, 4)
        in_maps.append({
            "xT": np.ascontiguousarray(x[b].T),
            "veT": np.ascontiguousarray(ve[b, :, g * HD:(g + 1) * HD].T),
            "crep": crep,
            "ssgn": ssgn,
            "wq": np.ascontiguousarray(Wq[:, g * 512:(g + 1) * 512]),
            "wk": np.ascontiguousarray(Wk[:, g * HD:(g + 1) * HD]),
            "wv": np.ascontiguousarray(Wv[:, g * HD:(g + 1) * HD]),
            "wg": np.ascontiguousarray(np.repeat(Wg[:, g:g + 1], 128, 1)),
            "wo": np.ascontiguousarray(Wo[g * 512:(g + 1) * 512, :]),
            "m_in": masks,
            "ones_in": ones128,
            "eye_in": eye128,
        })

    res = run_bass_kernel_spmd(nc, in_maps, core_ids=list(range(8)))
    parts = [res.results[c]["out"] for c in range(8)]
    out = np.stack([parts[0] + parts[1] + parts[2] + parts[3],
                    parts[4] + parts[5] + parts[6] + parts[7]])
    return out.astype(np.float32)
